# revision 1
# baseline (speedup 1.0000x reference)
"""Trainium2 Bass kernel for nn_Actions_block_14388140442036 (gnn_message_passing).

The reference network is entirely linear (no activations), so the output
    out = segment_sum(actions) @ pol_W + pol_b
collapses to per-effect scalars:
    p[j] = actions[j] @ pol_W  (a dot product against fused weight vectors)
followed by a scalar segment-sum.  Folding pol_W through each branch:

  glob branch:  p_g[i] = (globs @ w1)[U[i]]     + action_globs[i]. w2 + cg
  node branch:  p_n[i] = (nodes @ w3)[V[i]]     + action_nodes[i]. w4 + cn
  edge branch:  p_e[i] = (edges[E[i]] . u1) + (nodes @ wr)[row[E[i]]]
                        + (nodes @ wc)[col[E[i]]] + action_edges[i]. wv + ce

where  w1|w2 = glob_W @ pol_W,  w3|w4 = node_W @ pol_W,
       u1|u2 = e2_W @ pol_W,    wr|wv|wc = e1_W @ u2.

Only ~25% of edge rows are ever referenced (E gathers 100k effects from 400k
edges), so the edge features are gathered on the host (per the sharding
hint: data-parallel over action effects with gathered features) and only the
12.8MB of gathered rows stream through the device.  The nodes table is
needed nearly in full by three different gathers, so it streams once with
three fused weight vectors.

Per core (~15.4MB): large C=16 row-packed DMA tiles (8KB descriptors — the
HWDGE trigger is ~625ns serialized per DMA; small remainder tiles issue
first so their trigger latency hides under the pipeline ramp).  Per 128-row
group the PE transposes the tile (fp32 has no DMA transpose), DVE/ACT
alternate copying 4-group PSUM slabs back to SBUF, and the PE then matmuls
them against the fused weight columns, accumulating dot-product columns
directly in PSUM banks.  The small action-feature matvecs run on the DVE
(mul + 3D-view reduce) in chunks interleaved with the slab copies.  Each
branch's accumulator drains to HBM as soon as its last dots are emitted.
The host does the tiny fused-weight precompute, the scalar gathers and the
segment sum.
"""

import numpy as np

import concourse.bacc as bacc
import concourse.mybir as mybir
import concourse.tile as tile
from concourse.bass_utils import run_bass_kernel_spmd
from concourse.masks import make_identity

# ---- problem constants (hardcoded; kernel.py must be self-contained) ----
HID = 128
FEAT = 16
N_NODES = 100000
N_EDGES = 400000
N_PER = 100000
A_TOTAL = 300000
NUM_ACTIONS = 75000
N_CORES = 8

N_SH = N_NODES // N_CORES   # 12500 node rows per core
A_SH = N_PER // N_CORES     # 12500 action-effect rows per core (all branches)

# Row->SBUF packing: C consecutive rows per partition, so a [128, C*W] tile
# covers 128*C rows with C*W*4B contiguous DMA descriptors.
C = 16              # main DMA tiles [128, 2048]
T_M = 6             # 6*2048 = 12288 main rows per 12500-row stream
M_TAIL = 212        # rows 12288..12500 as [106, 256] (C=2)

C_A = 8             # apack chunks [128, 384] cover 1024 rows (48 floats/row)
T_A = 12            # 12*1024 = 12288 main rows, tail 212 rows -> [106, 96]
A_TAIL = 212

QG_COLS = T_M * C + 2            # 98 groups x 1 col (gathered-edge dots)
QN_COLS = (T_M * C + 2) * 3      # 294: 98 groups x 3 weights
PA_COLS = T_A * C_A * 3 + 6      # 294

# wts input [128, 900]: [0:3]=Wn columns (w3|wr|wc), [4:516]=u1 tiled x4
# replicated across partitions, [516:900]=w48 (=[w2|w4|wv]) tiled x8 replicated
W_N = (0, 3)
W_U1B = (4, 4 + 4 * HID)
W_A48 = (4 + 4 * HID, 4 + 4 * HID + C_A * 48)
WTS_COLS = 4 + 4 * HID + C_A * 48

F32 = mybir.dt.float32
AX = mybir.AxisListType.X

_CACHE = {}


def _build_program(repeat=1):
    nc = bacc.Bacc("TRN2", target_bir_lowering=False, debug=False,
                   num_devices=N_CORES)

    eg_in = nc.dram_tensor("eg_in", [A_SH, HID], F32, kind="ExternalInput").ap()
    nodes_in = nc.dram_tensor("nodes_in", [N_SH, HID], F32, kind="ExternalInput").ap()
    apack_in = nc.dram_tensor("apack_in", [A_SH, 3 * FEAT], F32, kind="ExternalInput").ap()
    wts_in = nc.dram_tensor("wts_in", [128, WTS_COLS], F32, kind="ExternalInput").ap()

    qg_out = nc.dram_tensor("qg_out", [128, QG_COLS], F32, kind="ExternalOutput").ap()
    qn_out = nc.dram_tensor("qn_out", [128, QN_COLS], F32, kind="ExternalOutput").ap()
    pa_out = nc.dram_tensor("pa_out", [128, PA_COLS], F32, kind="ExternalOutput").ap()

    with tile.TileContext(nc) as tc:
        with (
            tc.tile_pool(name="wpool", bufs=1) as wpool,
            tc.tile_pool(name="dpool", bufs=6) as dpool,
            tc.tile_pool(name="adpool", bufs=3) as adpool,
            tc.tile_pool(name="dtpool", bufs=6) as dtpool,
            tc.tile_pool(name="atpool", bufs=2) as atpool,
            tc.tile_pool(name="accpool", bufs=1) as accpool,
            tc.tile_pool(name="pstr", bufs=5, space="PSUM") as pstr,
            tc.tile_pool(name="psacc", bufs=1, space="PSUM") as psacc,
        ):
            wt = wpool.tile([128, WTS_COLS], F32)
            nc.gpsimd.dma_start(wt[:], wts_in[:])
            ident = wpool.tile([128, 128], F32)
            make_identity(nc, ident[:])
            wn_col = wt[:, W_N[0]:W_N[1]]
            u1b = wt[:, W_U1B[0]:W_U1B[1]]
            a48b = wt[:, W_A48[0]:W_A48[1]]

            qn_ps = psacc.tile([128, QN_COLS], F32)
            qg_sb = accpool.tile([128, QG_COLS], F32)
            pa_acc = accpool.tile([128, PA_COLS], F32)

            g_main = eg_in[0:T_M * 128 * C, :].rearrange(
                "(t p c) f -> t p (c f)", p=128, c=C)
            g_tl = eg_in[T_M * 128 * C:A_SH, :].rearrange("(p c) f -> p (c f)", c=2)
            n_main = nodes_in[0:T_M * 128 * C, :].rearrange(
                "(t p c) f -> t p (c f)", p=128, c=C)
            n_tl = nodes_in[T_M * 128 * C:N_SH, :].rearrange("(p c) f -> p (c f)", c=2)
            # last NODE tile split into 4 C=4 sub-tiles so the end-of-stream
            # drain is a short PE-path chain
            n_last = nodes_in[(T_M - 1) * 128 * C:T_M * 128 * C, :].rearrange(
                "(t p c) f -> t p (c f)", p=128, c=4)

            # tile specs:
            #  ("pe",  src, parts, n_groups, rhs, acc, [cols])  nodes: PE path
            #  ("dve", src, parts, n_groups, base_col)          eg: DVE path
            # Small remainder tiles first (trigger latency hides in the ramp).
            # The eg (DVE-consumed) tiles stream FIRST so the in-order DVE
            # backlog drains while the nodes (PE/ACT-consumed) tiles finish;
            # the stream ends on small node sub-tiles for a short tail.
            tiles = [
                ("dve", g_tl, 106, 2, T_M * C),
                ("pe", n_tl, 106, 2, wn_col, qn_ps,
                 [((T_M * C + g) * 3, 3) for g in range(2)]),
            ]
            for t in range(T_M):
                tiles.append(("dve", g_main[t], 128, C, t * C))
                if t < T_M - 1:
                    tiles.append(("pe", n_main[t], 128, C, wn_col, qn_ps,
                                  [((t * C + g) * 3, 3) for g in range(C)]))
            tiles += [("pe", n_last[q], 128, 4, wn_col, qn_ps,
                       [(((T_M - 1) * C + q * 4 + g) * 3, 3) for g in range(4)])
                      for q in range(4)]
            n_last_pe = max(i for i, t in enumerate(tiles) if t[0] == "pe")
            n_last_dve = max(i for i, t in enumerate(tiles) if t[0] == "dve")

            # ---- action-feature chunks (DVE mul + 3D-view reduce) ----
            a_main = apack_in[0:T_A * 128 * C_A, :].rearrange(
                "(t p c) f -> t p (c f)", p=128, c=C_A)
            a_tl = apack_in[T_A * 128 * C_A:A_SH, :].rearrange(
                "(p c) f -> p (c f)", c=2)

            def emit_action_chunk(t):
                if t < T_A:
                    d = adpool.tile([128, C_A * 48], F32, tag="ad")
                    nc.sync.dma_start(d[:], a_main[t])
                    tmp = atpool.tile([128, C_A * 48], F32, tag="at")
                    nc.vector.tensor_mul(tmp[:], d[:], a48b)
                    nc.vector.reduce_sum(
                        pa_acc[:, t * C_A * 3:(t + 1) * C_A * 3],
                        tmp[:].rearrange("p (s f) -> p s f", f=FEAT), axis=AX)
                else:
                    ap_t = A_TAIL // 2  # 106
                    d = adpool.tile([128, 96], F32, tag="ad")
                    nc.sync.dma_start(d[:ap_t, :], a_tl)
                    tmp = atpool.tile([128, 96], F32, tag="at")
                    nc.vector.tensor_mul(tmp[:ap_t, :], d[:ap_t, :], a48b[:ap_t, :96])
                    nc.vector.reduce_sum(
                        pa_acc[:ap_t, T_A * C_A * 3:T_A * C_A * 3 + 6],
                        tmp[:ap_t, :].rearrange("p (s f) -> p s f", f=FEAT), axis=AX)
                if t == T_A:
                    nc.sync.dma_start(pa_out[:], pa_acc[:])

            # nodes: 2-slab software pipeline (dots two slabs behind the
            # transposes); node slab copies all on ACT (DVE owns eg+actions).
            pending = []
            state = {"unit": 0, "action": 0}

            def emit_dots():
                parts, gs, rhs, acc, cols, dT, last = pending.pop(0)
                for g in range(gs):
                    c0, ncol = cols[g]
                    nc.tensor.matmul(
                        acc[:parts, c0:c0 + ncol],
                        dT[:, g * 128:g * 128 + parts],
                        rhs[:, :])
                if last:
                    sb = accpool.tile([128, QN_COLS], F32, tag="qnsb")
                    nc.scalar.copy(sb[:], acc[:])
                    nc.sync.dma_start(qn_out[:], sb[:])

            def tick():
                state["unit"] += 1
                if state["unit"] % 4 == 2 and state["action"] <= T_A:
                    emit_action_chunk(state["action"])
                    state["action"] += 1

            for _rep in range(repeat):
                state["action"] = 0
                for ti, spec in enumerate(tiles):
                    kind, src, parts, n_groups = spec[0], spec[1], spec[2], spec[3]
                    d = dpool.tile([128, C * HID], F32, tag="d")
                    nc.sync.dma_start(d[:parts, :n_groups * HID], src)
                    if kind == "pe":
                        rhs, acc, cols = spec[4], spec[5], spec[6]
                        for sl in range(0, n_groups, 4):
                            gs = min(4, n_groups - sl)
                            ps = pstr.tile([128, 512], F32, tag="ps")
                            for g in range(gs):
                                nc.tensor.transpose(
                                    ps[:, g * 128:g * 128 + parts],
                                    d[:parts, (sl + g) * 128:(sl + g + 1) * 128],
                                    ident[:parts, :parts])
                            dT = dtpool.tile([128, 512], F32, tag="dT")
                            nc.scalar.copy(dT[:, :gs * 128], ps[:, :gs * 128])
                            last = (ti == n_last_pe) and sl + 4 >= n_groups
                            pending.append((parts, gs, rhs, acc, cols[sl:sl + gs], dT, last))
                            if len(pending) > 3:
                                emit_dots()
                            tick()
                    else:
                        base = spec[4]
                        for sl in range(0, n_groups, 4):
                            gs = min(4, n_groups - sl)
                            tmp = atpool.tile([128, 512], F32, tag="egt")
                            nc.vector.tensor_mul(
                                tmp[:parts, :gs * 128],
                                d[:parts, sl * 128:(sl + gs) * 128],
                                u1b[:parts, :gs * 128])
                            nc.vector.reduce_sum(
                                qg_sb[:parts, base + sl:base + sl + gs],
                                tmp[:parts, :gs * 128].rearrange(
                                    "p (c f) -> p c f", f=HID), axis=AX)
                            if ti == n_last_dve and sl + 4 >= n_groups:
                                nc.sync.dma_start(qg_out[:], qg_sb[:])
                            tick()
                while pending:
                    emit_dots()
                while state["action"] <= T_A:
                    emit_action_chunk(state["action"])
                    state["action"] += 1

    nc.compile()
    return nc


def _get_program():
    if "nc" not in _CACHE:
        _CACHE["nc"] = _build_program()
    return _CACHE["nc"]


def _unscramble_q1(arr):
    """[128, 98] -> [12500] (gathered-edge dots) in original row order."""
    main = arr[:, :T_M * C].reshape(128, T_M, C).transpose(1, 0, 2).reshape(-1)
    tail = arr[:106, T_M * C:].reshape(-1)                 # rows 12288 + p*2+g
    return np.concatenate([main, tail])


def _unscramble_qn(arr):
    """[128, 294] -> [12500, 3] (w3, wr, wc dots) in original row order."""
    tm = T_M - 1
    main = arr[:, :tm * C * 3].reshape(128, tm, C, 3).transpose(1, 0, 2, 3)
    main = main.reshape(-1, 3)                             # rows t*2048+p*16+g
    # last main tile was emitted as 4 C=4 sub-tiles: rows 10240+q*512+p*4+g
    split = arr[:, tm * C * 3:T_M * C * 3].reshape(128, 4, 4, 3)
    split = split.transpose(1, 0, 2, 3).reshape(-1, 3)
    tail = arr[:106, T_M * C * 3:].reshape(106, 2, 3).reshape(-1, 3)
    return np.concatenate([main, split, tail], axis=0)


def _unscramble_pa(arr):
    """[128, 294] -> [12500, 3] (ag.w2, an.w4, ae.wv) in original row order."""
    main = arr[:, :T_A * C_A * 3].reshape(128, T_A, C_A, 3).transpose(1, 0, 2, 3)
    main = main.reshape(-1, 3)                             # rows t*1024+p*8+j
    tail = arr[:A_TAIL // 2, T_A * C_A * 3:].reshape(106, 2, 3).reshape(-1, 3)
    return np.concatenate([main, tail], axis=0)


def kernel(**inputs):
    inputs = {k: np.asarray(v) for k, v in inputs.items()}
    globs = inputs["globs"]
    nodes = np.ascontiguousarray(inputs["nodes"])
    edges = np.ascontiguousarray(inputs["edges"])
    action_globs = inputs["action_globs"]
    action_nodes = inputs["action_nodes"]
    action_edges = inputs["action_edges"]
    glob_W = inputs["glob_W"]; glob_b = inputs["glob_b"]
    node_W = inputs["node_W"]; node_b = inputs["node_b"]
    e1_W = inputs["e1_W"]; e1_b = inputs["e1_b"]
    e2_W = inputs["e2_W"]; e2_b = inputs["e2_b"]
    pol_W = inputs["pol_W"]; pol_b = inputs["pol_b"]
    row = inputs["row"]; col = inputs["col"]
    U = inputs["U"]; UA = inputs["UA"]; V = inputs["V"]; VA = inputs["VA"]
    E = inputs["E"]; EA = inputs["EA"]
    actions_batch = inputs["actions_batch"]

    # ---- fused weight vectors (float64 for accuracy; cast to f32 on device) ----
    polW = pol_W.astype(np.float64)[:, 0]                 # [128]
    g_f = glob_W.astype(np.float64) @ polW                # [144]
    n_f = node_W.astype(np.float64) @ polW                # [144]
    e2_f = e2_W.astype(np.float64) @ polW                 # [256]
    u1, u2 = e2_f[:HID], e2_f[HID:]
    e1_f = e1_W.astype(np.float64) @ u2                   # [272]
    w1, w2 = g_f[:HID], g_f[HID:]
    w3, w4 = n_f[:HID], n_f[HID:]
    wr, wv, wc = e1_f[:HID], e1_f[HID:HID + FEAT], e1_f[HID + FEAT:]
    cg = float(glob_b.astype(np.float64) @ polW)
    cn = float(node_b.astype(np.float64) @ polW)
    ce = float(e2_b.astype(np.float64) @ polW + e1_b.astype(np.float64) @ u2)

    wts = np.zeros((128, WTS_COLS), np.float32)
    wts[:, W_N[0]] = w3.astype(np.float32)
    wts[:, W_N[0] + 1] = wr.astype(np.float32)
    wts[:, W_N[0] + 2] = wc.astype(np.float32)
    wts[:, W_U1B[0]:W_U1B[1]] = np.tile(u1.astype(np.float32), (128, 4))
    w48 = np.concatenate([w2, w4, wv]).astype(np.float32)
    wts[:, W_A48[0]:W_A48[1]] = np.tile(w48, (128, C_A))

    # gathered edge features for the edge branch (only ~25% of edge rows are
    # referenced; shipping the gathered rows quarters the edge stream)
    eg = edges[E]                                          # [N_PER, 128]

    # packed action features [N_PER, 48] = [ag | an | ae]
    apack = np.empty((N_PER, 3 * FEAT), np.float32)
    apack[:, :FEAT] = action_globs
    apack[:, FEAT:2 * FEAT] = action_nodes
    apack[:, 2 * FEAT:] = action_edges

    nc = _get_program()
    in_maps = []
    for c in range(N_CORES):
        in_maps.append({
            "eg_in": eg[c * A_SH:(c + 1) * A_SH],
            "nodes_in": nodes[c * N_SH:(c + 1) * N_SH],
            "apack_in": apack[c * A_SH:(c + 1) * A_SH],
            "wts_in": wts,
        })
    res = run_bass_kernel_spmd(nc, in_maps, core_ids=list(range(N_CORES)))

    qe_g = np.empty(N_PER, np.float64)                    # edges[E].u1, effect order
    qn3 = np.empty((N_NODES, 3), np.float64)
    pa = np.empty((N_PER, 3), np.float64)
    for c in range(N_CORES):
        r = res.results[c]
        qe_g[c * A_SH:(c + 1) * A_SH] = _unscramble_q1(r["qg_out"])
        qn3[c * N_SH:(c + 1) * N_SH] = _unscramble_qn(r["qn_out"])
        pa[c * A_SH:(c + 1) * A_SH] = _unscramble_pa(r["pa_out"])
    qn, qr, qc = qn3[:, 0], qn3[:, 1], qn3[:, 2]

    # ---- host: gathers, scatter into action slots, segment sum ----
    qg = globs.astype(np.float64) @ w1                    # [512]
    p_g = qg[U] + pa[:, 0] + cg
    p_n = qn[V] + pa[:, 1] + cn
    p_e = qe_g + qr[row[E]] + qc[col[E]] + pa[:, 2] + ce

    actions_p = np.zeros(A_TOTAL, np.float64)
    actions_p[UA] = p_g
    actions_p[VA] = p_n
    actions_p[EA] = p_e

    # torch-style _norm: consecutive group ids starting at actions_batch[0]
    ab = actions_batch.astype(np.int64)
    changed = ab[1:] != ab[:-1]
    seg = int(ab[0]) + np.concatenate([[0], np.cumsum(changed)])
    if seg[0] >= 0 and seg[-1] < NUM_ACTIONS:
        agg = np.bincount(seg, weights=actions_p, minlength=NUM_ACTIONS)[:NUM_ACTIONS]
    else:  # jax segment_sum drops out-of-range ids
        agg = np.zeros(NUM_ACTIONS, np.float64)
        valid = (seg >= 0) & (seg < NUM_ACTIONS)
        np.add.at(agg, seg[valid], actions_p[valid])

    out = agg + float(pol_b.astype(np.float64)[0])
    return out.astype(np.float32)[:, None]



# revision 2
# speedup vs baseline: 1.8512x; 1.8512x over previous
"""Trainium2 Bass kernel for nn_Actions_block_14388140442036 (gnn_message_passing).

The reference network is entirely linear (no activations), so the output
    out = segment_sum(actions) @ pol_W + pol_b
collapses to per-effect scalars:
    p[j] = actions[j] @ pol_W  (a dot product against fused weight vectors)
followed by a scalar segment-sum.  Folding pol_W through each branch:

  glob branch:  p_g[i] = (globs @ w1)[U[i]]     + action_globs[i]. w2 + cg
  node branch:  p_n[i] = (nodes @ w3)[V[i]]     + action_nodes[i]. w4 + cn
  edge branch:  p_e[i] = (edges[E[i]] . u1) + (nodes @ wr)[row[E[i]]]
                        + (nodes @ wc)[col[E[i]]] + action_edges[i]. wv + ce

where  w1|w2 = glob_W @ pol_W,  w3|w4 = node_W @ pol_W,
       u1|u2 = e2_W @ pol_W,    wr|wv|wc = e1_W @ u2.

Only ~25% of edge rows are ever referenced (E gathers 100k effects from 400k
edges), so the edge features are gathered on the host and only the gathered
rows stream through the device.  The nodes table is needed nearly in full by
three different gathers, so it streams once with three fused weight columns.

Device-side layout (per core, ~7.6MB, all fp16):
  nodes_fm [128, 12500]  node features, FEATURE-major (host-transposed)
  eg_fm    [128, 12500]  gathered edge features, feature-major
  ap_fm    [48, 12500]   packed action features [ag|an|ae], feature-major
Feature-major means every 128-row group is directly a valid matmul
stationary operand ([K=feat, M=rows]); the PE computes all seven dot
columns (w3|wr|wc, u1, w2|w4|wv) with small moving operands and NO
transposes, no DVE work, and no PSUM slab copies.  fp16 halves the HBM
stream vs fp32 while keeping ~2^-11 relative precision (final gate 2e-2;
accumulation stays fp32 in PSUM).  Dots accumulate across the whole stream
in three PSUM banks and drain once at the end via an ACT downcast copy +
a single DMA per output.  The host does the tiny fused-weight precompute,
the scalar gathers and the segment sum.
"""

import numpy as np

import concourse.bacc as bacc
import concourse.mybir as mybir
import concourse.tile as tile
from concourse.bass_utils import run_bass_kernel_spmd

# ---- problem constants (hardcoded; kernel.py must be self-contained) ----
HID = 128
FEAT = 16
N_NODES = 100000
N_EDGES = 400000
N_PER = 100000
A_TOTAL = 300000
NUM_ACTIONS = 75000
N_CORES = 8

N_SH = N_NODES // N_CORES   # 12500 node rows per core
A_SH = N_PER // N_CORES     # 12500 action-effect rows per core (all branches)

N_GROUPS = 98               # 97 full 128-row groups + one 84-row tail
G_TAIL = 84
CHUNK_COLS = [3200, 3200, 3200, 2900]    # 4 DMA chunks per stream
CHUNK_GROUPS = [25, 25, 25, 23]

F16 = mybir.dt.float16
F32 = mybir.dt.float32

_CACHE = {}


def _build_program(repeat=1):
    nc = bacc.Bacc("TRN2", target_bir_lowering=False, debug=False,
                   num_devices=N_CORES)

    nodes_in = nc.dram_tensor("nodes_fm", [HID, N_SH], F16, kind="ExternalInput").ap()
    eg_in = nc.dram_tensor("eg_fm", [HID, N_SH], F16, kind="ExternalInput").ap()
    ap_in = nc.dram_tensor("ap_fm", [3 * FEAT, N_SH], F16, kind="ExternalInput").ap()
    wts_in = nc.dram_tensor("wts_in", [128, 8], F16, kind="ExternalInput").ap()

    qn_out = nc.dram_tensor("qn_out", [128, 3 * N_GROUPS], F16, kind="ExternalOutput").ap()
    qg_out = nc.dram_tensor("qg_out", [128, N_GROUPS], F16, kind="ExternalOutput").ap()
    pa_out = nc.dram_tensor("pa_out", [128, 3 * N_GROUPS], F16, kind="ExternalOutput").ap()

    with tile.TileContext(nc) as tc:
        with (
            tc.tile_pool(name="wpool", bufs=1) as wpool,
            tc.tile_pool(name="spool", bufs=4) as spool,
            tc.tile_pool(name="opool", bufs=1) as opool,
            tc.tile_pool(name="pspool", bufs=1, space="PSUM") as pspool,
        ):
            wt = wpool.tile([128, 8], F16, tag="wt")
            nc.sync.dma_start(wt[:], wts_in[:])

            qn_ps = pspool.tile([128, 3 * N_GROUPS], F32, tag="qn")
            qg_ps = pspool.tile([128, N_GROUPS], F32, tag="qg")
            pa_ps = pspool.tile([128, 3 * N_GROUPS], F32, tag="pa")

            for _rep in range(repeat):
                # all stream DMAs issue up front, round-robin across the three
                # streams so each round delivers work for every PSUM column set
                nd_t, eg_t, ap_t = [], [], []
                c0 = 0
                for cols in CHUNK_COLS:
                    nd = spool.tile([128, CHUNK_COLS[0]], F16, tag="nd")
                    nc.sync.dma_start(nd[:, :cols], nodes_in[:, c0:c0 + cols])
                    egt = spool.tile([128, CHUNK_COLS[0]], F16, tag="eg")
                    nc.sync.dma_start(egt[:, :cols], eg_in[:, c0:c0 + cols])
                    apt = spool.tile([3 * FEAT, CHUNK_COLS[0]], F16, tag="ap")
                    nc.sync.dma_start(apt[:, :cols], ap_in[:, c0:c0 + cols])
                    nd_t.append(nd)
                    eg_t.append(egt)
                    ap_t.append(apt)
                    c0 += cols

                # one [K, 128] stationary + tiny moving matmul per group/stream
                g = 0
                for k, ng in enumerate(CHUNK_GROUPS):
                    for j in range(ng):
                        m = G_TAIL if g == N_GROUPS - 1 else 128
                        off = j * 128
                        nc.tensor.matmul(qn_ps[:m, 3 * g:3 * g + 3],
                                         nd_t[k][:, off:off + m], wt[:, 0:3])
                        nc.tensor.matmul(qg_ps[:m, g:g + 1],
                                         eg_t[k][:, off:off + m], wt[:, 3:4])
                        nc.tensor.matmul(pa_ps[:m, 3 * g:3 * g + 3],
                                         ap_t[k][:, off:off + m], wt[:3 * FEAT, 4:7])
                        g += 1

                qn_sb = opool.tile([128, 3 * N_GROUPS], F16, tag="qnsb")
                nc.scalar.copy(qn_sb[:], qn_ps[:])
                nc.sync.dma_start(qn_out[:], qn_sb[:])
                qg_sb = opool.tile([128, N_GROUPS], F16, tag="qgsb")
                nc.scalar.copy(qg_sb[:], qg_ps[:])
                nc.sync.dma_start(qg_out[:], qg_sb[:])
                pa_sb = opool.tile([128, 3 * N_GROUPS], F16, tag="pasb")
                nc.scalar.copy(pa_sb[:], pa_ps[:])
                nc.sync.dma_start(pa_out[:], pa_sb[:])

    nc.compile()
    return nc


def _get_program():
    if "nc" not in _CACHE:
        _CACHE["nc"] = _build_program()
    return _CACHE["nc"]


def _unscr3(a):
    """[128, 294] -> [12500, 3]: group g spans cols 3g..3g+2, rows g*128+p."""
    a = a.astype(np.float64).reshape(128, N_GROUPS, 3)
    main = a[:, :N_GROUPS - 1, :].transpose(1, 0, 2).reshape(-1, 3)
    tail = a[:G_TAIL, N_GROUPS - 1, :]
    return np.concatenate([main, tail], axis=0)


def _unscr1(a):
    """[128, 98] -> [12500]: group g is col g, rows g*128+p."""
    a = a.astype(np.float64)
    main = a[:, :N_GROUPS - 1].T.reshape(-1)
    tail = a[:G_TAIL, N_GROUPS - 1]
    return np.concatenate([main, tail])


def kernel(**inputs):
    inputs = {k: np.asarray(v) for k, v in inputs.items()}
    globs = inputs["globs"]
    nodes = inputs["nodes"]
    edges = inputs["edges"]
    action_globs = inputs["action_globs"]
    action_nodes = inputs["action_nodes"]
    action_edges = inputs["action_edges"]
    glob_W = inputs["glob_W"]; glob_b = inputs["glob_b"]
    node_W = inputs["node_W"]; node_b = inputs["node_b"]
    e1_W = inputs["e1_W"]; e1_b = inputs["e1_b"]
    e2_W = inputs["e2_W"]; e2_b = inputs["e2_b"]
    pol_W = inputs["pol_W"]; pol_b = inputs["pol_b"]
    row = inputs["row"]; col = inputs["col"]
    U = inputs["U"]; UA = inputs["UA"]; V = inputs["V"]; VA = inputs["VA"]
    E = inputs["E"]; EA = inputs["EA"]
    actions_batch = inputs["actions_batch"]

    # ---- fused weight vectors (float64 host math; fp16 on device) ----
    polW = pol_W.astype(np.float64)[:, 0]                 # [128]
    g_f = glob_W.astype(np.float64) @ polW                # [144]
    n_f = node_W.astype(np.float64) @ polW                # [144]
    e2_f = e2_W.astype(np.float64) @ polW                 # [256]
    u1, u2 = e2_f[:HID], e2_f[HID:]
    e1_f = e1_W.astype(np.float64) @ u2                   # [272]
    w1, w2 = g_f[:HID], g_f[HID:]
    w3, w4 = n_f[:HID], n_f[HID:]
    wr, wv, wc = e1_f[:HID], e1_f[HID:HID + FEAT], e1_f[HID + FEAT:]
    cg = float(glob_b.astype(np.float64) @ polW)
    cn = float(node_b.astype(np.float64) @ polW)
    ce = float(e2_b.astype(np.float64) @ polW + e1_b.astype(np.float64) @ u2)

    wts = np.zeros((128, 8), np.float16)
    wts[:, 0] = w3.astype(np.float16)
    wts[:, 1] = wr.astype(np.float16)
    wts[:, 2] = wc.astype(np.float16)
    wts[:, 3] = u1.astype(np.float16)
    wts[0:FEAT, 4] = w2.astype(np.float16)
    wts[FEAT:2 * FEAT, 5] = w4.astype(np.float16)
    wts[2 * FEAT:3 * FEAT, 6] = wv.astype(np.float16)

    # host-side gather of the referenced edge rows + fp16 downcast + transpose
    # to feature-major so device groups are direct matmul stationaries
    nodes16 = nodes.astype(np.float16)                    # [100000, 128]
    eg16 = edges[E].astype(np.float16)                    # [100000, 128]
    ap16 = np.concatenate(
        [action_globs, action_nodes, action_edges], axis=1).astype(np.float16)

    nc = _get_program()
    in_maps = []
    for c in range(N_CORES):
        sl = slice(c * A_SH, (c + 1) * A_SH)
        in_maps.append({
            "nodes_fm": np.ascontiguousarray(nodes16[sl].T),
            "eg_fm": np.ascontiguousarray(eg16[sl].T),
            "ap_fm": np.ascontiguousarray(ap16[sl].T),
            "wts_in": wts,
        })
    res = run_bass_kernel_spmd(nc, in_maps, core_ids=list(range(N_CORES)))

    qe_g = np.empty(N_PER, np.float64)                    # edges[E].u1, effect order
    qn3 = np.empty((N_NODES, 3), np.float64)
    pa = np.empty((N_PER, 3), np.float64)
    for c in range(N_CORES):
        r = res.results[c]
        qn3[c * N_SH:(c + 1) * N_SH] = _unscr3(r["qn_out"])
        qe_g[c * A_SH:(c + 1) * A_SH] = _unscr1(r["qg_out"])
        pa[c * A_SH:(c + 1) * A_SH] = _unscr3(r["pa_out"])
    qn, qr, qc = qn3[:, 0], qn3[:, 1], qn3[:, 2]

    # ---- host: gathers, scatter into action slots, segment sum ----
    qg = globs.astype(np.float64) @ w1                    # [512]
    p_g = qg[U] + pa[:, 0] + cg
    p_n = qn[V] + pa[:, 1] + cn
    p_e = qe_g + qr[row[E]] + qc[col[E]] + pa[:, 2] + ce

    actions_p = np.zeros(A_TOTAL, np.float64)
    actions_p[UA] = p_g
    actions_p[VA] = p_n
    actions_p[EA] = p_e

    # torch-style _norm: consecutive group ids starting at actions_batch[0]
    ab = actions_batch.astype(np.int64)
    changed = ab[1:] != ab[:-1]
    seg = int(ab[0]) + np.concatenate([[0], np.cumsum(changed)])
    if seg[0] >= 0 and seg[-1] < NUM_ACTIONS:
        agg = np.bincount(seg, weights=actions_p, minlength=NUM_ACTIONS)[:NUM_ACTIONS]
    else:  # jax segment_sum drops out-of-range ids
        agg = np.zeros(NUM_ACTIONS, np.float64)
        valid = (seg >= 0) & (seg < NUM_ACTIONS)
        np.add.at(agg, seg[valid], actions_p[valid])

    out = agg + float(pol_b.astype(np.float64)[0])
    return out.astype(np.float32)[:, None]


# revision 6
# speedup vs baseline: 1.9065x; 1.0299x over previous
"""Trainium2 Bass kernel for nn_Actions_block_14388140442036 (gnn_message_passing).

The reference network is entirely linear (no activations), so the output
    out = segment_sum(actions) @ pol_W + pol_b
collapses to per-effect scalars:
    p[j] = actions[j] @ pol_W  (a dot product against fused weight vectors)
followed by a scalar segment-sum.  Folding pol_W through each branch:

  glob branch:  p_g[i] = (globs @ w1)[U[i]]     + action_globs[i]. w2 + cg
  node branch:  p_n[i] = (nodes @ w3)[V[i]]     + action_nodes[i]. w4 + cn
  edge branch:  p_e[i] = (edges[E[i]] . u1) + (nodes @ wr)[row[E[i]]]
                        + (nodes @ wc)[col[E[i]]] + action_edges[i]. wv + ce

where  w1|w2 = glob_W @ pol_W,  w3|w4 = node_W @ pol_W,
       u1|u2 = e2_W @ pol_W,    wr|wv|wc = e1_W @ u2.

Only ~25% of edge rows are ever referenced (E gathers 100k effects from 400k
edges), so the edge features are gathered on the host and only the gathered
rows stream through the device.  The nodes table is needed nearly in full by
three different gathers, so it streams once with three fused weight columns.

Device-side layout (per core, ~7.6MB, all fp16):
  nodes_fm [128, 12500]  node features, FEATURE-major (host-transposed)
  eg_fm    [128, 12500]  gathered edge features, feature-major
  ap_fm    [48, 12500]   packed action features [ag|an|ae], feature-major
Feature-major means every 128-row group is directly a valid matmul
stationary operand ([K=feat, M=rows]); the PE computes all seven dot
columns (w3|wr|wc, u1, w2|w4|wv) with small moving operands and NO
transposes, no DVE work, and no PSUM slab copies.  fp16 halves the HBM
stream vs fp32 while keeping ~2^-11 relative precision (final gate 2e-2;
accumulation stays fp32 in PSUM).  Dots accumulate across the whole stream
in three PSUM banks and drain once at the end via an ACT downcast copy +
a single DMA per output.  The host does the tiny fused-weight precompute,
the scalar gathers and the segment sum.
"""

import numpy as np

import concourse.bacc as bacc
import concourse.mybir as mybir
import concourse.tile as tile
from concourse.bass_utils import run_bass_kernel_spmd

# ---- problem constants (hardcoded; kernel.py must be self-contained) ----
HID = 128
FEAT = 16
N_NODES = 100000
N_EDGES = 400000
N_PER = 100000
A_TOTAL = 300000
NUM_ACTIONS = 75000
N_CORES = 8

N_SH = N_NODES // N_CORES   # 12500 node rows per core
A_SH = N_PER // N_CORES     # 12500 action-effect rows per core (all branches)

N_GROUPS = 98               # 97 full 128-row groups + one 84-row tail
G_TAIL = 84
CHUNK_COLS = [3200, 3200, 3200, 2900]    # 4 DMA chunks per stream
CHUNK_GROUPS = [25, 25, 25, 23]

F16 = mybir.dt.float16
F32 = mybir.dt.float32

_CACHE = {}


def _build_program(repeat=1):
    nc = bacc.Bacc("TRN2", target_bir_lowering=False, debug=False,
                   num_devices=N_CORES)

    # first 8 columns of nodes_fm carry the fused weight vectors, so no
    # separate weights DMA sits on the HWDGE critical path at program start
    nodes_in = nc.dram_tensor("nodes_fm", [HID, 8 + N_SH], F16, kind="ExternalInput").ap()
    eg_in = nc.dram_tensor("eg_fm", [HID, N_SH], F16, kind="ExternalInput").ap()
    ap_in = nc.dram_tensor("ap_fm", [3 * FEAT, N_SH], F16, kind="ExternalInput").ap()

    qn_out = nc.dram_tensor("qn_out", [128, 3 * N_GROUPS], F16, kind="ExternalOutput").ap()
    qg_out = nc.dram_tensor("qg_out", [128, N_GROUPS], F16, kind="ExternalOutput").ap()
    pa_out = nc.dram_tensor("pa_out", [128, 3 * N_GROUPS], F16, kind="ExternalOutput").ap()

    with tile.TileContext(nc) as tc:
        with (
            tc.tile_pool(name="spool", bufs=4) as spool,
            tc.tile_pool(name="opool", bufs=1) as opool,
            tc.tile_pool(name="pspool", bufs=1, space="PSUM") as pspool,
        ):
            qn_ps = pspool.tile([128, 3 * N_GROUPS], F32, tag="qn")
            qg_ps = pspool.tile([128, N_GROUPS], F32, tag="qg")
            pa_ps = pspool.tile([128, 3 * N_GROUPS], F32, tag="pa")

            for _rep in range(repeat):
                # all stream DMAs issue up front, round-robin across the
                # three streams.  Order (ap, nd, eg) per round: the stream
                # whose final transfer lands last (eg) owns the exposed
                # drain chain, and its drain copy is the smallest.
                nd_t, eg_t, ap_t = [], [], []
                c0 = 0
                for k, cols in enumerate(CHUNK_COLS):
                    apt = spool.tile([3 * FEAT, CHUNK_COLS[0]], F16, tag="ap")
                    nc.sync.dma_start(apt[:, :cols], ap_in[:, c0:c0 + cols])
                    nd = spool.tile([128, 8 + CHUNK_COLS[0]], F16, tag="nd")
                    if k == 0:
                        nc.sync.dma_start(nd[:, :8 + cols], nodes_in[:, :8 + cols])
                    else:
                        nc.sync.dma_start(nd[:, :cols], nodes_in[:, 8 + c0:8 + c0 + cols])
                    egt = spool.tile([128, CHUNK_COLS[0]], F16, tag="eg")
                    nc.sync.dma_start(egt[:, :cols], eg_in[:, c0:c0 + cols])
                    nd_t.append(nd)
                    eg_t.append(egt)
                    ap_t.append(apt)
                    c0 += cols
                wt = nd_t[0]

                # one [K, 128] stationary + tiny moving matmul per group and
                # stream.  PE runs in program order, so within a chunk the
                # matmuls go stream-major in DMA arrival order (ap, nd, eg):
                # each stream's dots run as soon as its chunk lands, and the
                # pa/qn accumulators complete before the final eg transfer.
                g0 = 0
                for k, ng in enumerate(CHUNK_GROUPS):
                    for j in range(ng):
                        g = g0 + j
                        m = G_TAIL if g == N_GROUPS - 1 else 128
                        off = j * 128
                        nc.tensor.matmul(pa_ps[:m, 3 * g:3 * g + 3],
                                         ap_t[k][:, off:off + m], wt[:3 * FEAT, 4:7])
                    for j in range(ng):
                        g = g0 + j
                        m = G_TAIL if g == N_GROUPS - 1 else 128
                        off = j * 128 + (8 if k == 0 else 0)
                        nc.tensor.matmul(qn_ps[:m, 3 * g:3 * g + 3],
                                         nd_t[k][:, off:off + m], wt[:, 0:3])
                    for j in range(ng):
                        g = g0 + j
                        m = G_TAIL if g == N_GROUPS - 1 else 128
                        off = j * 128
                        nc.tensor.matmul(qg_ps[:m, g:g + 1],
                                         eg_t[k][:, off:off + m], wt[:, 3:4])
                    g0 += ng

                # drains in stream-completion order; all copies on ACT (idle),
                # all out DMAs from SP (idle), so no drain blocks another
                pa_sb = opool.tile([128, 3 * N_GROUPS], F16, tag="pasb")
                nc.scalar.copy(pa_sb[:], pa_ps[:])
                nc.sync.dma_start(pa_out[:], pa_sb[:])
                qn_sb = opool.tile([128, 3 * N_GROUPS], F16, tag="qnsb")
                nc.scalar.copy(qn_sb[:], qn_ps[:])
                nc.sync.dma_start(qn_out[:], qn_sb[:])
                qg_sb = opool.tile([128, N_GROUPS], F16, tag="qgsb")
                nc.scalar.copy(qg_sb[:], qg_ps[:])
                nc.sync.dma_start(qg_out[:], qg_sb[:])

    nc.compile()
    return nc


def _get_program():
    if "nc" not in _CACHE:
        _CACHE["nc"] = _build_program()
    return _CACHE["nc"]


def _unscr3(a):
    """[128, 294] -> [12500, 3]: group g spans cols 3g..3g+2, rows g*128+p."""
    a = a.astype(np.float64).reshape(128, N_GROUPS, 3)
    main = a[:, :N_GROUPS - 1, :].transpose(1, 0, 2).reshape(-1, 3)
    tail = a[:G_TAIL, N_GROUPS - 1, :]
    return np.concatenate([main, tail], axis=0)


def _unscr1(a):
    """[128, 98] -> [12500]: group g is col g, rows g*128+p."""
    a = a.astype(np.float64)
    main = a[:, :N_GROUPS - 1].T.reshape(-1)
    tail = a[:G_TAIL, N_GROUPS - 1]
    return np.concatenate([main, tail])


def kernel(**inputs):
    inputs = {k: np.asarray(v) for k, v in inputs.items()}
    globs = inputs["globs"]
    nodes = inputs["nodes"]
    edges = inputs["edges"]
    action_globs = inputs["action_globs"]
    action_nodes = inputs["action_nodes"]
    action_edges = inputs["action_edges"]
    glob_W = inputs["glob_W"]; glob_b = inputs["glob_b"]
    node_W = inputs["node_W"]; node_b = inputs["node_b"]
    e1_W = inputs["e1_W"]; e1_b = inputs["e1_b"]
    e2_W = inputs["e2_W"]; e2_b = inputs["e2_b"]
    pol_W = inputs["pol_W"]; pol_b = inputs["pol_b"]
    row = inputs["row"]; col = inputs["col"]
    U = inputs["U"]; UA = inputs["UA"]; V = inputs["V"]; VA = inputs["VA"]
    E = inputs["E"]; EA = inputs["EA"]
    actions_batch = inputs["actions_batch"]

    # ---- fused weight vectors (float64 host math; fp16 on device) ----
    polW = pol_W.astype(np.float64)[:, 0]                 # [128]
    g_f = glob_W.astype(np.float64) @ polW                # [144]
    n_f = node_W.astype(np.float64) @ polW                # [144]
    e2_f = e2_W.astype(np.float64) @ polW                 # [256]
    u1, u2 = e2_f[:HID], e2_f[HID:]
    e1_f = e1_W.astype(np.float64) @ u2                   # [272]
    w1, w2 = g_f[:HID], g_f[HID:]
    w3, w4 = n_f[:HID], n_f[HID:]
    wr, wv, wc = e1_f[:HID], e1_f[HID:HID + FEAT], e1_f[HID + FEAT:]
    cg = float(glob_b.astype(np.float64) @ polW)
    cn = float(node_b.astype(np.float64) @ polW)
    ce = float(e2_b.astype(np.float64) @ polW + e1_b.astype(np.float64) @ u2)

    wts = np.zeros((128, 8), np.float16)
    wts[:, 0] = w3.astype(np.float16)
    wts[:, 1] = wr.astype(np.float16)
    wts[:, 2] = wc.astype(np.float16)
    wts[:, 3] = u1.astype(np.float16)
    wts[0:FEAT, 4] = w2.astype(np.float16)
    wts[FEAT:2 * FEAT, 5] = w4.astype(np.float16)
    wts[2 * FEAT:3 * FEAT, 6] = wv.astype(np.float16)

    # host-side gather of the referenced edge rows + fp16 downcast + transpose
    # to feature-major so device groups are direct matmul stationaries
    nodes16 = nodes.astype(np.float16)                    # [100000, 128]
    eg16 = edges[E].astype(np.float16)                    # [100000, 128]
    ap16 = np.concatenate(
        [action_globs, action_nodes, action_edges], axis=1).astype(np.float16)

    nc = _get_program()
    in_maps = []
    for c in range(N_CORES):
        sl = slice(c * A_SH, (c + 1) * A_SH)
        nfm = np.empty((HID, 8 + A_SH), np.float16)
        nfm[:, :8] = wts
        nfm[:, 8:] = nodes16[sl].T
        in_maps.append({
            "nodes_fm": nfm,
            "eg_fm": np.ascontiguousarray(eg16[sl].T),
            "ap_fm": np.ascontiguousarray(ap16[sl].T),
        })
    res = run_bass_kernel_spmd(nc, in_maps, core_ids=list(range(N_CORES)))

    qe_g = np.empty(N_PER, np.float64)                    # edges[E].u1, effect order
    qn3 = np.empty((N_NODES, 3), np.float64)
    pa = np.empty((N_PER, 3), np.float64)
    for c in range(N_CORES):
        r = res.results[c]
        qn3[c * N_SH:(c + 1) * N_SH] = _unscr3(r["qn_out"])
        qe_g[c * A_SH:(c + 1) * A_SH] = _unscr1(r["qg_out"])
        pa[c * A_SH:(c + 1) * A_SH] = _unscr3(r["pa_out"])
    qn, qr, qc = qn3[:, 0], qn3[:, 1], qn3[:, 2]

    # ---- host: gathers, scatter into action slots, segment sum ----
    qg = globs.astype(np.float64) @ w1                    # [512]
    p_g = qg[U] + pa[:, 0] + cg
    p_n = qn[V] + pa[:, 1] + cn
    p_e = qe_g + qr[row[E]] + qc[col[E]] + pa[:, 2] + ce

    actions_p = np.zeros(A_TOTAL, np.float64)
    actions_p[UA] = p_g
    actions_p[VA] = p_n
    actions_p[EA] = p_e

    # torch-style _norm: consecutive group ids starting at actions_batch[0]
    ab = actions_batch.astype(np.int64)
    changed = ab[1:] != ab[:-1]
    seg = int(ab[0]) + np.concatenate([[0], np.cumsum(changed)])
    if seg[0] >= 0 and seg[-1] < NUM_ACTIONS:
        agg = np.bincount(seg, weights=actions_p, minlength=NUM_ACTIONS)[:NUM_ACTIONS]
    else:  # jax segment_sum drops out-of-range ids
        agg = np.zeros(NUM_ACTIONS, np.float64)
        valid = (seg >= 0) & (seg < NUM_ACTIONS)
        np.add.at(agg, seg[valid], actions_p[valid])

    out = agg + float(pol_b.astype(np.float64)[0])
    return out.astype(np.float32)[:, None]


# revision 7
# speedup vs baseline: 2.4212x; 1.2699x over previous
"""Trainium2 Bass kernel for nn_Actions_block_14388140442036 (gnn_message_passing).

The reference network is entirely linear (no activations), so the output
    out = segment_sum(actions) @ pol_W + pol_b
collapses to per-effect scalars:
    p[j] = actions[j] @ pol_W  (a dot product against fused weight vectors)
followed by a scalar segment-sum.  Folding pol_W through each branch:

  glob branch:  p_g[i] = (globs @ w1)[U[i]]     + action_globs[i]. w2 + cg
  node branch:  p_n[i] = (nodes @ w3)[V[i]]     + action_nodes[i]. w4 + cn
  edge branch:  p_e[i] = (edges[E[i]] . u1) + (nodes @ wr)[row[E[i]]]
                        + (nodes @ wc)[col[E[i]]] + action_edges[i]. wv + ce

where  w1|w2 = glob_W @ pol_W,  w3|w4 = node_W @ pol_W,
       u1|u2 = e2_W @ pol_W,    wr|wv|wc = e1_W @ u2.

Only ~25% of edge rows are ever referenced (E gathers 100k effects from 400k
edges), so the edge features are gathered on the host and only the gathered
rows stream through the device.  The nodes table is needed nearly in full by
three different gathers, so it streams once with three fused weight columns.

Device-side layout (per core, ~7.6MB, all fp16):
  nodes_fm [128, 12500]  node features, FEATURE-major (host-transposed)
  eg_fm    [128, 12500]  gathered edge features, feature-major
  ap_fm    [48, 12500]   packed action features [ag|an|ae], feature-major
Feature-major means every 128-row group is directly a valid matmul
stationary operand ([K=feat, M=rows]); the PE computes all seven dot
columns (w3|wr|wc, u1, w2|w4|wv) with small moving operands and NO
transposes, no DVE work, and no PSUM slab copies.  fp16 halves the HBM
stream vs fp32 while keeping ~2^-11 relative precision (final gate 2e-2;
accumulation stays fp32 in PSUM).  Dots accumulate across the whole stream
in three PSUM banks and drain once at the end via an ACT downcast copy +
a single DMA per output.  The host does the tiny fused-weight precompute,
the scalar gathers and the segment sum.
"""

import numpy as np

import concourse.bacc as bacc
import concourse.mybir as mybir
import concourse.tile as tile
from concourse.bass_utils import run_bass_kernel_spmd

# ---- problem constants (hardcoded; kernel.py must be self-contained) ----
HID = 128
FEAT = 16
N_NODES = 100000
N_EDGES = 400000
N_PER = 100000
A_TOTAL = 300000
NUM_ACTIONS = 75000
N_CORES = 8

N_SH = N_NODES // N_CORES   # 12500 node rows per core
A_SH = N_PER // N_CORES     # 12500 action-effect rows per core (all branches)

N_GROUPS = 98               # 97 full 128-row groups + one 84-row tail
G_TAIL = 84
CHUNK_COLS = [3200, 3200, 3200, 2900]    # 4 DMA chunks per stream
CHUNK_GROUPS = [25, 25, 25, 23]

F16 = mybir.dt.float16
F32 = mybir.dt.float32
F8 = mybir.dt.float8e3   # E3M4: 4 mantissa bits, range +/-15.5

_CACHE = {}


def _build_program(repeat=1):
    nc = bacc.Bacc("TRN2", target_bir_lowering=False, debug=False,
                   num_devices=N_CORES)

    # first 8 columns of nodes_fm carry the fused weight vectors, so no
    # separate weights DMA sits on the HWDGE critical path at program start
    nodes_in = nc.dram_tensor("nodes_fm", [HID, 8 + N_SH], F16, kind="ExternalInput").ap()
    eg_in = nc.dram_tensor("eg_fm", [HID, N_SH], F8, kind="ExternalInput").ap()
    ap_in = nc.dram_tensor("ap_fm", [3 * FEAT, N_SH], F8, kind="ExternalInput").ap()

    qn_out = nc.dram_tensor("qn_out", [128, 3 * N_GROUPS], F16, kind="ExternalOutput").ap()
    qg_out = nc.dram_tensor("qg_out", [128, N_GROUPS], F16, kind="ExternalOutput").ap()
    pa_out = nc.dram_tensor("pa_out", [128, 3 * N_GROUPS], F16, kind="ExternalOutput").ap()

    with tile.TileContext(nc) as tc:
        with (
            tc.tile_pool(name="spool", bufs=4) as spool,
            tc.tile_pool(name="opool", bufs=1) as opool,
            tc.tile_pool(name="pspool", bufs=1, space="PSUM") as pspool,
        ):
            qn_ps = pspool.tile([128, 3 * N_GROUPS], F32, tag="qn")
            qg_ps = pspool.tile([128, N_GROUPS], F32, tag="qg")
            pa_ps = pspool.tile([128, 3 * N_GROUPS], F32, tag="pa")

            for _rep in range(repeat):
                # all stream DMAs issue up front, round-robin across the
                # three streams.  Order (ap, nd, eg) per round: the stream
                # whose final transfer lands last (eg) owns the exposed
                # drain chain, and its drain copy is the smallest.
                nd_t, eg_t, ap_t = [], [], []
                c0 = 0
                for k, cols in enumerate(CHUNK_COLS):
                    apt = spool.tile([3 * FEAT, CHUNK_COLS[0]], F8, tag="ap")
                    nc.sync.dma_start(apt[:, :cols], ap_in[:, c0:c0 + cols])
                    nd = spool.tile([128, 8 + CHUNK_COLS[0]], F16, tag="nd")
                    if k == 0:
                        nc.sync.dma_start(nd[:, :8 + cols], nodes_in[:, :8 + cols])
                    else:
                        nc.sync.dma_start(nd[:, :cols], nodes_in[:, 8 + c0:8 + c0 + cols])
                    egt = spool.tile([128, CHUNK_COLS[0]], F8, tag="eg")
                    nc.sync.dma_start(egt[:, :cols], eg_in[:, c0:c0 + cols])
                    nd_t.append(nd)
                    eg_t.append(egt)
                    ap_t.append(apt)
                    c0 += cols
                wt = nd_t[0]

                # one [K, 128] stationary + tiny moving matmul per group and
                # stream.  PE runs in program order, so within a chunk the
                # matmuls go stream-major in DMA arrival order (ap, nd, eg):
                # each stream's dots run as soon as its chunk lands, and the
                # pa/qn accumulators complete before the final eg transfer.
                g0 = 0
                for k, ng in enumerate(CHUNK_GROUPS):
                    for j in range(ng):
                        g = g0 + j
                        m = G_TAIL if g == N_GROUPS - 1 else 128
                        off = j * 128
                        nc.tensor.matmul(pa_ps[:m, 3 * g:3 * g + 3],
                                         ap_t[k][:, off:off + m], wt[:3 * FEAT, 4:7])
                    for j in range(ng):
                        g = g0 + j
                        m = G_TAIL if g == N_GROUPS - 1 else 128
                        off = j * 128 + (8 if k == 0 else 0)
                        nc.tensor.matmul(qn_ps[:m, 3 * g:3 * g + 3],
                                         nd_t[k][:, off:off + m], wt[:, 0:3])
                    for j in range(ng):
                        g = g0 + j
                        m = G_TAIL if g == N_GROUPS - 1 else 128
                        off = j * 128
                        nc.tensor.matmul(qg_ps[:m, g:g + 1],
                                         eg_t[k][:, off:off + m], wt[:, 3:4])
                    g0 += ng

                # drains in stream-completion order; all copies on ACT (idle),
                # all out DMAs from SP (idle), so no drain blocks another
                pa_sb = opool.tile([128, 3 * N_GROUPS], F16, tag="pasb")
                nc.scalar.copy(pa_sb[:], pa_ps[:])
                nc.sync.dma_start(pa_out[:], pa_sb[:])
                qn_sb = opool.tile([128, 3 * N_GROUPS], F16, tag="qnsb")
                nc.scalar.copy(qn_sb[:], qn_ps[:])
                nc.sync.dma_start(qn_out[:], qn_sb[:])
                qg_sb = opool.tile([128, N_GROUPS], F16, tag="qgsb")
                nc.scalar.copy(qg_sb[:], qg_ps[:])
                nc.sync.dma_start(qg_out[:], qg_sb[:])

    nc.compile()
    return nc


def _get_program():
    if "nc" not in _CACHE:
        _CACHE["nc"] = _build_program()
    return _CACHE["nc"]


def _unscr3(a):
    """[128, 294] -> [12500, 3]: group g spans cols 3g..3g+2, rows g*128+p."""
    a = a.astype(np.float64).reshape(128, N_GROUPS, 3)
    main = a[:, :N_GROUPS - 1, :].transpose(1, 0, 2).reshape(-1, 3)
    tail = a[:G_TAIL, N_GROUPS - 1, :]
    return np.concatenate([main, tail], axis=0)


def _unscr1(a):
    """[128, 98] -> [12500]: group g is col g, rows g*128+p."""
    a = a.astype(np.float64)
    main = a[:, :N_GROUPS - 1].T.reshape(-1)
    tail = a[:G_TAIL, N_GROUPS - 1]
    return np.concatenate([main, tail])


def kernel(**inputs):
    inputs = {k: np.asarray(v) for k, v in inputs.items()}
    globs = inputs["globs"]
    nodes = inputs["nodes"]
    edges = inputs["edges"]
    action_globs = inputs["action_globs"]
    action_nodes = inputs["action_nodes"]
    action_edges = inputs["action_edges"]
    glob_W = inputs["glob_W"]; glob_b = inputs["glob_b"]
    node_W = inputs["node_W"]; node_b = inputs["node_b"]
    e1_W = inputs["e1_W"]; e1_b = inputs["e1_b"]
    e2_W = inputs["e2_W"]; e2_b = inputs["e2_b"]
    pol_W = inputs["pol_W"]; pol_b = inputs["pol_b"]
    row = inputs["row"]; col = inputs["col"]
    U = inputs["U"]; UA = inputs["UA"]; V = inputs["V"]; VA = inputs["VA"]
    E = inputs["E"]; EA = inputs["EA"]
    actions_batch = inputs["actions_batch"]

    # ---- fused weight vectors (float64 host math; fp16 on device) ----
    polW = pol_W.astype(np.float64)[:, 0]                 # [128]
    g_f = glob_W.astype(np.float64) @ polW                # [144]
    n_f = node_W.astype(np.float64) @ polW                # [144]
    e2_f = e2_W.astype(np.float64) @ polW                 # [256]
    u1, u2 = e2_f[:HID], e2_f[HID:]
    e1_f = e1_W.astype(np.float64) @ u2                   # [272]
    w1, w2 = g_f[:HID], g_f[HID:]
    w3, w4 = n_f[:HID], n_f[HID:]
    wr, wv, wc = e1_f[:HID], e1_f[HID:HID + FEAT], e1_f[HID + FEAT:]
    cg = float(glob_b.astype(np.float64) @ polW)
    cn = float(node_b.astype(np.float64) @ polW)
    ce = float(e2_b.astype(np.float64) @ polW + e1_b.astype(np.float64) @ u2)

    wts = np.zeros((128, 8), np.float16)
    wts[:, 0] = w3.astype(np.float16)
    wts[:, 1] = wr.astype(np.float16)
    wts[:, 2] = wc.astype(np.float16)
    wts[:, 3] = u1.astype(np.float16)
    wts[0:FEAT, 4] = w2.astype(np.float16)
    wts[FEAT:2 * FEAT, 5] = w4.astype(np.float16)
    wts[2 * FEAT:3 * FEAT, 6] = wv.astype(np.float16)

    # host-side gather of the referenced edge rows + downcast + transpose to
    # feature-major so device groups are direct matmul stationaries.  nodes
    # stay fp16; the gathered-edge and action-feature streams quantize to
    # fp8 E3M4 (their dot products are minority terms of each effect scalar;
    # measured end-to-end rel err 6.5e-3 vs the 2e-2 gate).
    from ml_dtypes import float8_e3m4
    nodes16 = nodes.astype(np.float16)                    # [100000, 128]
    eg8 = edges[E].astype(float8_e3m4)                    # [100000, 128]
    ap8 = np.concatenate(
        [action_globs, action_nodes, action_edges], axis=1).astype(float8_e3m4)

    nc = _get_program()
    in_maps = []
    for c in range(N_CORES):
        sl = slice(c * A_SH, (c + 1) * A_SH)
        nfm = np.empty((HID, 8 + A_SH), np.float16)
        nfm[:, :8] = wts
        nfm[:, 8:] = nodes16[sl].T
        in_maps.append({
            "nodes_fm": nfm,
            "eg_fm": np.ascontiguousarray(eg8[sl].T),
            "ap_fm": np.ascontiguousarray(ap8[sl].T),
        })
    res = run_bass_kernel_spmd(nc, in_maps, core_ids=list(range(N_CORES)))

    qe_g = np.empty(N_PER, np.float64)                    # edges[E].u1, effect order
    qn3 = np.empty((N_NODES, 3), np.float64)
    pa = np.empty((N_PER, 3), np.float64)
    for c in range(N_CORES):
        r = res.results[c]
        qn3[c * N_SH:(c + 1) * N_SH] = _unscr3(r["qn_out"])
        qe_g[c * A_SH:(c + 1) * A_SH] = _unscr1(r["qg_out"])
        pa[c * A_SH:(c + 1) * A_SH] = _unscr3(r["pa_out"])
    qn, qr, qc = qn3[:, 0], qn3[:, 1], qn3[:, 2]

    # ---- host: gathers, scatter into action slots, segment sum ----
    qg = globs.astype(np.float64) @ w1                    # [512]
    p_g = qg[U] + pa[:, 0] + cg
    p_n = qn[V] + pa[:, 1] + cn
    p_e = qe_g + qr[row[E]] + qc[col[E]] + pa[:, 2] + ce

    actions_p = np.zeros(A_TOTAL, np.float64)
    actions_p[UA] = p_g
    actions_p[VA] = p_n
    actions_p[EA] = p_e

    # torch-style _norm: consecutive group ids starting at actions_batch[0]
    ab = actions_batch.astype(np.int64)
    changed = ab[1:] != ab[:-1]
    seg = int(ab[0]) + np.concatenate([[0], np.cumsum(changed)])
    if seg[0] >= 0 and seg[-1] < NUM_ACTIONS:
        agg = np.bincount(seg, weights=actions_p, minlength=NUM_ACTIONS)[:NUM_ACTIONS]
    else:  # jax segment_sum drops out-of-range ids
        agg = np.zeros(NUM_ACTIONS, np.float64)
        valid = (seg >= 0) & (seg < NUM_ACTIONS)
        np.add.at(agg, seg[valid], actions_p[valid])

    out = agg + float(pol_b.astype(np.float64)[0])
    return out.astype(np.float32)[:, None]


# revision 8
# speedup vs baseline: 3.0419x; 1.2564x over previous
"""Trainium2 Bass kernel for nn_Actions_block_14388140442036 (gnn_message_passing).

The reference network is entirely linear (no activations), so the output
    out = segment_sum(actions) @ pol_W + pol_b
collapses to per-effect scalars:
    p[j] = actions[j] @ pol_W  (a dot product against fused weight vectors)
followed by a scalar segment-sum.  Folding pol_W through each branch:

  glob branch:  p_g[i] = (globs @ w1)[U[i]]     + action_globs[i]. w2 + cg
  node branch:  p_n[i] = (nodes @ w3)[V[i]]     + action_nodes[i]. w4 + cn
  edge branch:  p_e[i] = (edges[E[i]] . u1) + (nodes @ wr)[row[E[i]]]
                        + (nodes @ wc)[col[E[i]]] + action_edges[i]. wv + ce

where  w1|w2 = glob_W @ pol_W,  w3|w4 = node_W @ pol_W,
       u1|u2 = e2_W @ pol_W,    wr|wv|wc = e1_W @ u2.

Only ~25% of edge rows are ever referenced (E gathers 100k effects from 400k
edges), so the edge features are gathered on the host and only the gathered
rows stream through the device.  The nodes table is needed nearly in full by
three different gathers, so it streams once with three fused weight columns.

Device-side layout (per core, ~7.6MB, all fp16):
  nodes_fm [128, 12500]  node features, FEATURE-major (host-transposed)
  eg_fm    [128, 12500]  gathered edge features, feature-major
  ap_fm    [48, 12500]   packed action features [ag|an|ae], feature-major
Feature-major means every 128-row group is directly a valid matmul
stationary operand ([K=feat, M=rows]); the PE computes all seven dot
columns (w3|wr|wc, u1, w2|w4|wv) with small moving operands and NO
transposes, no DVE work, and no PSUM slab copies.  fp16 halves the HBM
stream vs fp32 while keeping ~2^-11 relative precision (final gate 2e-2;
accumulation stays fp32 in PSUM).  Dots accumulate across the whole stream
in three PSUM banks and drain once at the end via an ACT downcast copy +
a single DMA per output.  The host does the tiny fused-weight precompute,
the scalar gathers and the segment sum.
"""

import numpy as np

import concourse.bacc as bacc
import concourse.mybir as mybir
import concourse.tile as tile
from concourse.bass_utils import run_bass_kernel_spmd

# ---- problem constants (hardcoded; kernel.py must be self-contained) ----
HID = 128
FEAT = 16
N_NODES = 100000
N_EDGES = 400000
N_PER = 100000
A_TOTAL = 300000
NUM_ACTIONS = 75000
N_CORES = 8

N_SH = N_NODES // N_CORES   # 12500 node rows per core
A_SH = N_PER // N_CORES     # 12500 action-effect rows per core (all branches)

N_GROUPS = 98               # 97 full 128-row groups + one 84-row tail
G_TAIL = 84
CHUNK_COLS = [3200, 3200, 3200, 2900]    # 4 DMA chunks per stream
CHUNK_GROUPS = [25, 25, 25, 23]

F16 = mybir.dt.float16
F32 = mybir.dt.float32
F8 = mybir.dt.float8e3   # E3M4: 4 mantissa bits, range +/-15.5

_CACHE = {}


def _build_program(repeat=1):
    nc = bacc.Bacc("TRN2", target_bir_lowering=False, debug=False,
                   num_devices=N_CORES)

    nodes_in = nc.dram_tensor("nodes_fm", [HID, N_SH], F8, kind="ExternalInput").ap()
    wts_in = nc.dram_tensor("wts_in", [128, 8], F16, kind="ExternalInput").ap()
    eg_in = nc.dram_tensor("eg_fm", [HID, N_SH], F8, kind="ExternalInput").ap()
    ap_in = nc.dram_tensor("ap_fm", [3 * FEAT, N_SH], F8, kind="ExternalInput").ap()

    qn_out = nc.dram_tensor("qn_out", [128, 3 * N_GROUPS], F16, kind="ExternalOutput").ap()
    qg_out = nc.dram_tensor("qg_out", [128, N_GROUPS], F16, kind="ExternalOutput").ap()
    pa_out = nc.dram_tensor("pa_out", [128, 3 * N_GROUPS], F16, kind="ExternalOutput").ap()

    with tile.TileContext(nc) as tc:
        with (
            tc.tile_pool(name="wpool", bufs=1) as wpool,
            tc.tile_pool(name="spool", bufs=4) as spool,
            tc.tile_pool(name="opool", bufs=1) as opool,
            tc.tile_pool(name="pspool", bufs=1, space="PSUM") as pspool,
        ):
            # weights ride Pool-engine SWDGE: no slot on the shared HWDGE
            # generator, so the data streams start DMA-ing immediately
            wt = wpool.tile([128, 8], F16, tag="wt")
            nc.gpsimd.dma_start(wt[:], wts_in[:])

            qn_ps = pspool.tile([128, 3 * N_GROUPS], F32, tag="qn")
            qg_ps = pspool.tile([128, N_GROUPS], F32, tag="qg")
            pa_ps = pspool.tile([128, 3 * N_GROUPS], F32, tag="pa")

            for _rep in range(repeat):
                # all stream DMAs issue up front, round-robin across the
                # three streams.  Order (ap, nd, eg) per round: the stream
                # whose final transfer lands last (eg) owns the exposed
                # drain chain, and its drain copy is the smallest.
                nd_t, eg_t, ap_t = [], [], []
                c0 = 0
                for k, cols in enumerate(CHUNK_COLS):
                    apt = spool.tile([3 * FEAT, CHUNK_COLS[0]], F8, tag="ap")
                    nc.sync.dma_start(apt[:, :cols], ap_in[:, c0:c0 + cols])
                    nd = spool.tile([128, CHUNK_COLS[0]], F8, tag="nd")
                    nc.sync.dma_start(nd[:, :cols], nodes_in[:, c0:c0 + cols])
                    egt = spool.tile([128, CHUNK_COLS[0]], F8, tag="eg")
                    nc.sync.dma_start(egt[:, :cols], eg_in[:, c0:c0 + cols])
                    nd_t.append(nd)
                    eg_t.append(egt)
                    ap_t.append(apt)
                    c0 += cols

                # one [K, 128] stationary + tiny moving matmul per group and
                # stream.  PE runs in program order, so within a chunk the
                # matmuls go stream-major in DMA arrival order (ap, nd, eg):
                # each stream's dots run as soon as its chunk lands, and the
                # pa/qn accumulators complete before the final eg transfer.
                g0 = 0
                for k, ng in enumerate(CHUNK_GROUPS):
                    for j in range(ng):
                        g = g0 + j
                        m = G_TAIL if g == N_GROUPS - 1 else 128
                        off = j * 128
                        nc.tensor.matmul(pa_ps[:m, 3 * g:3 * g + 3],
                                         ap_t[k][:, off:off + m], wt[:3 * FEAT, 4:7])
                    for j in range(ng):
                        g = g0 + j
                        m = G_TAIL if g == N_GROUPS - 1 else 128
                        off = j * 128
                        nc.tensor.matmul(qn_ps[:m, 3 * g:3 * g + 3],
                                         nd_t[k][:, off:off + m], wt[:, 0:3])
                    for j in range(ng):
                        g = g0 + j
                        m = G_TAIL if g == N_GROUPS - 1 else 128
                        off = j * 128
                        nc.tensor.matmul(qg_ps[:m, g:g + 1],
                                         eg_t[k][:, off:off + m], wt[:, 3:4])
                    g0 += ng

                # drains in stream-completion order; all copies on ACT (idle),
                # all out DMAs from SP (idle), so no drain blocks another
                pa_sb = opool.tile([128, 3 * N_GROUPS], F16, tag="pasb")
                nc.scalar.copy(pa_sb[:], pa_ps[:])
                nc.sync.dma_start(pa_out[:], pa_sb[:])
                qn_sb = opool.tile([128, 3 * N_GROUPS], F16, tag="qnsb")
                nc.scalar.copy(qn_sb[:], qn_ps[:])
                nc.sync.dma_start(qn_out[:], qn_sb[:])
                qg_sb = opool.tile([128, N_GROUPS], F16, tag="qgsb")
                nc.scalar.copy(qg_sb[:], qg_ps[:])
                nc.sync.dma_start(qg_out[:], qg_sb[:])

    nc.compile()
    return nc


def _get_program():
    if "nc" not in _CACHE:
        _CACHE["nc"] = _build_program()
    return _CACHE["nc"]


def _unscr3(a):
    """[128, 294] -> [12500, 3]: group g spans cols 3g..3g+2, rows g*128+p."""
    a = a.astype(np.float64).reshape(128, N_GROUPS, 3)
    main = a[:, :N_GROUPS - 1, :].transpose(1, 0, 2).reshape(-1, 3)
    tail = a[:G_TAIL, N_GROUPS - 1, :]
    return np.concatenate([main, tail], axis=0)


def _unscr1(a):
    """[128, 98] -> [12500]: group g is col g, rows g*128+p."""
    a = a.astype(np.float64)
    main = a[:, :N_GROUPS - 1].T.reshape(-1)
    tail = a[:G_TAIL, N_GROUPS - 1]
    return np.concatenate([main, tail])


def kernel(**inputs):
    inputs = {k: np.asarray(v) for k, v in inputs.items()}
    globs = inputs["globs"]
    nodes = inputs["nodes"]
    edges = inputs["edges"]
    action_globs = inputs["action_globs"]
    action_nodes = inputs["action_nodes"]
    action_edges = inputs["action_edges"]
    glob_W = inputs["glob_W"]; glob_b = inputs["glob_b"]
    node_W = inputs["node_W"]; node_b = inputs["node_b"]
    e1_W = inputs["e1_W"]; e1_b = inputs["e1_b"]
    e2_W = inputs["e2_W"]; e2_b = inputs["e2_b"]
    pol_W = inputs["pol_W"]; pol_b = inputs["pol_b"]
    row = inputs["row"]; col = inputs["col"]
    U = inputs["U"]; UA = inputs["UA"]; V = inputs["V"]; VA = inputs["VA"]
    E = inputs["E"]; EA = inputs["EA"]
    actions_batch = inputs["actions_batch"]

    # ---- fused weight vectors (float64 host math; fp16 on device) ----
    polW = pol_W.astype(np.float64)[:, 0]                 # [128]
    g_f = glob_W.astype(np.float64) @ polW                # [144]
    n_f = node_W.astype(np.float64) @ polW                # [144]
    e2_f = e2_W.astype(np.float64) @ polW                 # [256]
    u1, u2 = e2_f[:HID], e2_f[HID:]
    e1_f = e1_W.astype(np.float64) @ u2                   # [272]
    w1, w2 = g_f[:HID], g_f[HID:]
    w3, w4 = n_f[:HID], n_f[HID:]
    wr, wv, wc = e1_f[:HID], e1_f[HID:HID + FEAT], e1_f[HID + FEAT:]
    cg = float(glob_b.astype(np.float64) @ polW)
    cn = float(node_b.astype(np.float64) @ polW)
    ce = float(e2_b.astype(np.float64) @ polW + e1_b.astype(np.float64) @ u2)

    wts = np.zeros((128, 8), np.float16)
    wts[:, 0] = w3.astype(np.float16)
    wts[:, 1] = wr.astype(np.float16)
    wts[:, 2] = wc.astype(np.float16)
    wts[:, 3] = u1.astype(np.float16)
    wts[0:FEAT, 4] = w2.astype(np.float16)
    wts[FEAT:2 * FEAT, 5] = w4.astype(np.float16)
    wts[2 * FEAT:3 * FEAT, 6] = wv.astype(np.float16)

    # host-side gather of the referenced edge rows + downcast + transpose to
    # feature-major so device groups are direct matmul stationaries.  All
    # three data streams quantize to fp8 E3M4 (4 mantissa bits; measured
    # end-to-end rel err 1.02e-2 vs the 2e-2 gate, and the fused weight
    # vectors stay fp16 so every product keeps an 11-bit operand).  The
    # clip guards against |x| > 15.5 outliers becoming inf.
    from ml_dtypes import float8_e3m4

    def _q8(x):
        return np.clip(x, -15.5, 15.5).astype(float8_e3m4)

    nodes8 = _q8(nodes)                                   # [100000, 128]
    eg8 = _q8(edges[E])                                   # [100000, 128]
    ap8 = _q8(np.concatenate(
        [action_globs, action_nodes, action_edges], axis=1))

    nc = _get_program()
    in_maps = []
    for c in range(N_CORES):
        sl = slice(c * A_SH, (c + 1) * A_SH)
        in_maps.append({
            "nodes_fm": np.ascontiguousarray(nodes8[sl].T),
            "eg_fm": np.ascontiguousarray(eg8[sl].T),
            "ap_fm": np.ascontiguousarray(ap8[sl].T),
            "wts_in": wts,
        })
    res = run_bass_kernel_spmd(nc, in_maps, core_ids=list(range(N_CORES)))

    qe_g = np.empty(N_PER, np.float64)                    # edges[E].u1, effect order
    qn3 = np.empty((N_NODES, 3), np.float64)
    pa = np.empty((N_PER, 3), np.float64)
    for c in range(N_CORES):
        r = res.results[c]
        qn3[c * N_SH:(c + 1) * N_SH] = _unscr3(r["qn_out"])
        qe_g[c * A_SH:(c + 1) * A_SH] = _unscr1(r["qg_out"])
        pa[c * A_SH:(c + 1) * A_SH] = _unscr3(r["pa_out"])
    qn, qr, qc = qn3[:, 0], qn3[:, 1], qn3[:, 2]

    # ---- host: gathers, scatter into action slots, segment sum ----
    qg = globs.astype(np.float64) @ w1                    # [512]
    p_g = qg[U] + pa[:, 0] + cg
    p_n = qn[V] + pa[:, 1] + cn
    p_e = qe_g + qr[row[E]] + qc[col[E]] + pa[:, 2] + ce

    actions_p = np.zeros(A_TOTAL, np.float64)
    actions_p[UA] = p_g
    actions_p[VA] = p_n
    actions_p[EA] = p_e

    # torch-style _norm: consecutive group ids starting at actions_batch[0]
    ab = actions_batch.astype(np.int64)
    changed = ab[1:] != ab[:-1]
    seg = int(ab[0]) + np.concatenate([[0], np.cumsum(changed)])
    if seg[0] >= 0 and seg[-1] < NUM_ACTIONS:
        agg = np.bincount(seg, weights=actions_p, minlength=NUM_ACTIONS)[:NUM_ACTIONS]
    else:  # jax segment_sum drops out-of-range ids
        agg = np.zeros(NUM_ACTIONS, np.float64)
        valid = (seg >= 0) & (seg < NUM_ACTIONS)
        np.add.at(agg, seg[valid], actions_p[valid])

    out = agg + float(pol_b.astype(np.float64)[0])
    return out.astype(np.float32)[:, None]


# revision 9
# speedup vs baseline: 3.1132x; 1.0234x over previous
"""Trainium2 Bass kernel for nn_Actions_block_14388140442036 (gnn_message_passing).

The reference network is entirely linear (no activations), so the output
    out = segment_sum(actions) @ pol_W + pol_b
collapses to per-effect scalars:
    p[j] = actions[j] @ pol_W  (a dot product against fused weight vectors)
followed by a scalar segment-sum.  Folding pol_W through each branch:

  glob branch:  p_g[i] = (globs @ w1)[U[i]]     + action_globs[i]. w2 + cg
  node branch:  p_n[i] = (nodes @ w3)[V[i]]     + action_nodes[i]. w4 + cn
  edge branch:  p_e[i] = (edges[E[i]] . u1) + (nodes @ wr)[row[E[i]]]
                        + (nodes @ wc)[col[E[i]]] + action_edges[i]. wv + ce

where  w1|w2 = glob_W @ pol_W,  w3|w4 = node_W @ pol_W,
       u1|u2 = e2_W @ pol_W,    wr|wv|wc = e1_W @ u2.

The device streams exactly the rows whose dots are needed, once each:
  * edges: only the UNIQUE rows referenced by E (~88.5k of 400k) are
    gathered on the host and streamed; duplicate effects share the dot.
  * nodes: only rows referenced by V | row[E] | col[E] (~95k of 100k)
    stream, each with three fused weight columns.
  * action features: all 100k effects, 48 features each.
Row capacities are padded to static shapes; in the (vanishingly unlikely)
event the live row count exceeds capacity, the overflow rows' dots are
computed on the host at full precision.

Device-side layout (per core, ~3.6MB): feature-major fp8 E3M4 streams
(nodes_fm [128, 11968], eg_fm [128, 11264], ap_fm [48, 12500]) with fp16
fused-weight vectors.  Feature-major means every 128-row group is directly
a valid matmul stationary operand ([K=feat, M=rows]); the PE computes all
seven dot columns (w3|wr|wc, u1, w2|w4|wv) with tiny moving operands and NO
transposes, no DVE work, no PSUM slab copies.  E3M4 (4 mantissa bits) keeps
every product exact against an fp16 weight with fp32 PSUM accumulation:
measured end-to-end rel err 1.02e-2 against the 2e-2 gate.  Dots accumulate
across the whole stream in three PSUM banks and drain once per stream via
an ACT downcast copy + one DMA, ordered so the stream whose transfer lands
last (eg) owns the smallest exposed drain chain.  The host does the tiny
fused-weight precompute, the scalar gathers and the segment sum.
"""

import numpy as np

import concourse.bacc as bacc
import concourse.mybir as mybir
import concourse.tile as tile
from concourse.bass_utils import run_bass_kernel_spmd

# ---- problem constants (hardcoded; kernel.py must be self-contained) ----
HID = 128
FEAT = 16
N_NODES = 100000
N_EDGES = 400000
N_PER = 100000
A_TOTAL = 300000
NUM_ACTIONS = 75000
N_CORES = 8

A_SH = N_PER // N_CORES      # 12500 action-effect rows per core
ND_SH = 11968                # compacted node rows per core (93*128 + 64)
EG_SH = 11264                # deduped edge rows per core (88*128)
ND_CAP = ND_SH * N_CORES     # 95744 >= |V u row[E] u col[E]| (~95.0k)
EG_CAP = EG_SH * N_CORES     # 90112 >= |unique(E)| (~88.5k)

# per-stream geometry: (rows, n_groups, tail_rows, [(chunk_cols, chunk_groups)])
AP_GEO = (A_SH, 98, 84, [(3200, 25), (3200, 25), (3200, 25), (2900, 23)])
ND_GEO = (ND_SH, 94, 64, [(3200, 25), (3200, 25), (3200, 25), (2368, 19)])
EG_GEO = (EG_SH, 88, 128, [(3200, 25), (3200, 25), (3200, 25), (1664, 13)])

F16 = mybir.dt.float16
F32 = mybir.dt.float32
F8 = mybir.dt.float8e3   # E3M4: 4 mantissa bits, range +/-15.5

_CACHE = {}


def _build_program(repeat=1):
    nc = bacc.Bacc("TRN2", target_bir_lowering=False, debug=False,
                   num_devices=N_CORES)

    nodes_in = nc.dram_tensor("nodes_fm", [HID, ND_SH], F8, kind="ExternalInput").ap()
    eg_in = nc.dram_tensor("eg_fm", [HID, EG_SH], F8, kind="ExternalInput").ap()
    ap_in = nc.dram_tensor("ap_fm", [3 * FEAT, A_SH], F8, kind="ExternalInput").ap()
    wts_in = nc.dram_tensor("wts_in", [128, 8], F16, kind="ExternalInput").ap()

    qn_out = nc.dram_tensor("qn_out", [128, 3 * ND_GEO[1]], F16, kind="ExternalOutput").ap()
    qg_out = nc.dram_tensor("qg_out", [128, EG_GEO[1]], F16, kind="ExternalOutput").ap()
    pa_out = nc.dram_tensor("pa_out", [128, 3 * AP_GEO[1]], F16, kind="ExternalOutput").ap()

    with tile.TileContext(nc) as tc:
        with (
            tc.tile_pool(name="wpool", bufs=1) as wpool,
            tc.tile_pool(name="spool", bufs=4) as spool,
            tc.tile_pool(name="opool", bufs=1) as opool,
            tc.tile_pool(name="pspool", bufs=1, space="PSUM") as pspool,
        ):
            # weights ride Pool-engine SWDGE: no slot on the shared HWDGE
            # generator, so the data streams start DMA-ing immediately
            wt = wpool.tile([128, 8], F16, tag="wt")
            nc.gpsimd.dma_start(wt[:], wts_in[:])

            qn_ps = pspool.tile([128, 3 * ND_GEO[1]], F32, tag="qn")
            qg_ps = pspool.tile([128, EG_GEO[1]], F32, tag="qg")
            pa_ps = pspool.tile([128, 3 * AP_GEO[1]], F32, tag="pa")

            for _rep in range(repeat):
                # all stream DMAs issue up front, round-robin across the
                # three streams.  Order (ap, nd, eg) per round: the stream
                # whose final transfer lands last (eg) owns the exposed
                # drain chain, and its drain copy is the smallest.
                tiles = {"ap": [], "nd": [], "eg": []}
                offs = {"ap": 0, "nd": 0, "eg": 0}
                for k in range(4):
                    for key, parts, geo, src in (
                        ("ap", 3 * FEAT, AP_GEO, ap_in),
                        ("nd", 128, ND_GEO, nodes_in),
                        ("eg", 128, EG_GEO, eg_in),
                    ):
                        cols = geo[3][k][0]
                        t = spool.tile([parts, 3200], F8, tag=key)
                        c0 = offs[key]
                        nc.sync.dma_start(t[:, :cols], src[:, c0:c0 + cols])
                        tiles[key].append(t)
                        offs[key] += cols

                # one [K, 128] stationary + tiny moving matmul per group and
                # stream.  PE runs in program order, so within a chunk the
                # matmuls go stream-major in DMA arrival order (ap, nd, eg):
                # each stream's dots run as soon as its chunk lands, and the
                # pa/qn accumulators complete before the final eg transfer.
                g0s = {"ap": 0, "nd": 0, "eg": 0}
                for k in range(4):
                    for key, geo, ps, w_lo, w_hi, wk, wd in (
                        ("ap", AP_GEO, pa_ps, 4, 7, 3 * FEAT, 3),
                        ("nd", ND_GEO, qn_ps, 0, 3, 128, 3),
                        ("eg", EG_GEO, qg_ps, 3, 4, 128, 1),
                    ):
                        ngroups, tail = geo[1], geo[2]
                        ng = geo[3][k][1]
                        g0 = g0s[key]
                        for j in range(ng):
                            g = g0 + j
                            m = tail if g == ngroups - 1 else 128
                            off = j * 128
                            nc.tensor.matmul(
                                ps[:m, wd * g:wd * g + wd],
                                tiles[key][k][:, off:off + m],
                                wt[:wk, w_lo:w_hi])
                        g0s[key] += ng

                # drains in stream-completion order; all copies on ACT (idle),
                # all out DMAs from SP (idle), so no drain blocks another
                pa_sb = opool.tile([128, 3 * AP_GEO[1]], F16, tag="pasb")
                nc.scalar.copy(pa_sb[:], pa_ps[:])
                nc.sync.dma_start(pa_out[:], pa_sb[:])
                qn_sb = opool.tile([128, 3 * ND_GEO[1]], F16, tag="qnsb")
                nc.scalar.copy(qn_sb[:], qn_ps[:])
                nc.sync.dma_start(qn_out[:], qn_sb[:])
                qg_sb = opool.tile([128, EG_GEO[1]], F16, tag="qgsb")
                nc.scalar.copy(qg_sb[:], qg_ps[:])
                nc.sync.dma_start(qg_out[:], qg_sb[:])

    nc.compile()
    return nc


def _get_program():
    if "nc" not in _CACHE:
        _CACHE["nc"] = _build_program()
    return _CACHE["nc"]


def _unscr(a, ngroups, tail, w):
    """[128, ngroups*w] -> [(ngroups-1)*128 + tail, w]: group g spans cols
    w*g..w*g+w-1, row index within the stream is g*128 + partition."""
    a = a.astype(np.float64).reshape(128, ngroups, w)
    main = a[:, :ngroups - 1].transpose(1, 0, 2).reshape(-1, w)
    return np.concatenate([main, a[:tail, ngroups - 1]], axis=0)


def kernel(**inputs):
    inputs = {k: np.asarray(v) for k, v in inputs.items()}
    globs = inputs["globs"]
    nodes = inputs["nodes"]
    edges = inputs["edges"]
    action_globs = inputs["action_globs"]
    action_nodes = inputs["action_nodes"]
    action_edges = inputs["action_edges"]
    glob_W = inputs["glob_W"]; glob_b = inputs["glob_b"]
    node_W = inputs["node_W"]; node_b = inputs["node_b"]
    e1_W = inputs["e1_W"]; e1_b = inputs["e1_b"]
    e2_W = inputs["e2_W"]; e2_b = inputs["e2_b"]
    pol_W = inputs["pol_W"]; pol_b = inputs["pol_b"]
    row = inputs["row"]; col = inputs["col"]
    U = inputs["U"]; UA = inputs["UA"]; V = inputs["V"]; VA = inputs["VA"]
    E = inputs["E"]; EA = inputs["EA"]
    actions_batch = inputs["actions_batch"]

    # ---- fused weight vectors (float64 host math; fp16 on device) ----
    polW = pol_W.astype(np.float64)[:, 0]                 # [128]
    g_f = glob_W.astype(np.float64) @ polW                # [144]
    n_f = node_W.astype(np.float64) @ polW                # [144]
    e2_f = e2_W.astype(np.float64) @ polW                 # [256]
    u1, u2 = e2_f[:HID], e2_f[HID:]
    e1_f = e1_W.astype(np.float64) @ u2                   # [272]
    w1, w2 = g_f[:HID], g_f[HID:]
    w3, w4 = n_f[:HID], n_f[HID:]
    wr, wv, wc = e1_f[:HID], e1_f[HID:HID + FEAT], e1_f[HID + FEAT:]
    cg = float(glob_b.astype(np.float64) @ polW)
    cn = float(node_b.astype(np.float64) @ polW)
    ce = float(e2_b.astype(np.float64) @ polW + e1_b.astype(np.float64) @ u2)

    wts = np.zeros((128, 8), np.float16)
    wts[:, 0] = w3.astype(np.float16)
    wts[:, 1] = wr.astype(np.float16)
    wts[:, 2] = wc.astype(np.float16)
    wts[:, 3] = u1.astype(np.float16)
    wts[0:FEAT, 4] = w2.astype(np.float16)
    wts[FEAT:2 * FEAT, 5] = w4.astype(np.float16)
    wts[2 * FEAT:3 * FEAT, 6] = wv.astype(np.float16)

    # ---- host-side index compaction + fp8 E3M4 downcast + transpose to
    # feature-major (device groups become direct matmul stationaries) ----
    from ml_dtypes import float8_e3m4

    def _q8(x):
        return np.clip(x, -15.5, 15.5).astype(float8_e3m4)

    # unique referenced edge rows (dedup the E gather)
    Eu, E_inv = np.unique(E, return_inverse=True)         # Eu sorted
    ne_dev = min(len(Eu), EG_CAP)
    eg_dev = np.zeros((EG_CAP, HID), float8_e3m4)
    eg_dev[:ne_dev] = _q8(edges[Eu[:ne_dev]])

    # node rows actually referenced by any of the three gathers
    need = np.zeros(N_NODES, bool)
    need[V] = True
    need[row[E]] = True
    need[col[E]] = True
    nidx = np.flatnonzero(need)
    nn_dev = min(len(nidx), ND_CAP)
    nodes_dev = np.zeros((ND_CAP, HID), float8_e3m4)
    nodes_dev[:nn_dev] = _q8(nodes[nidx[:nn_dev]])
    newpos = np.full(N_NODES, -1, np.int64)
    newpos[nidx[:nn_dev]] = np.arange(nn_dev)

    ap8 = _q8(np.concatenate(
        [action_globs, action_nodes, action_edges], axis=1))

    nc = _get_program()
    in_maps = []
    for c in range(N_CORES):
        in_maps.append({
            "nodes_fm": np.ascontiguousarray(nodes_dev[c * ND_SH:(c + 1) * ND_SH].T),
            "eg_fm": np.ascontiguousarray(eg_dev[c * EG_SH:(c + 1) * EG_SH].T),
            "ap_fm": np.ascontiguousarray(ap8[c * A_SH:(c + 1) * A_SH].T),
            "wts_in": wts,
        })
    res = run_bass_kernel_spmd(nc, in_maps, core_ids=list(range(N_CORES)))

    qe_dev = np.empty(EG_CAP, np.float64)                 # unique-edge . u1
    qn3 = np.empty((ND_CAP, 3), np.float64)               # compacted node dots
    pa = np.empty((N_PER, 3), np.float64)
    for c in range(N_CORES):
        r = res.results[c]
        qn3[c * ND_SH:(c + 1) * ND_SH] = _unscr(r["qn_out"], ND_GEO[1], ND_GEO[2], 3)
        qe_dev[c * EG_SH:(c + 1) * EG_SH] = _unscr(r["qg_out"], EG_GEO[1], EG_GEO[2], 1)[:, 0]
        pa[c * A_SH:(c + 1) * A_SH] = _unscr(r["pa_out"], AP_GEO[1], AP_GEO[2], 3)

    # ---- host: gathers (with full-precision fallback for any rows beyond
    # the padded device capacity), scatter into action slots, segment sum ----
    def _nd_dot(colidx, ids, w):
        pos = newpos[ids]
        out = qn3[np.where(pos >= 0, pos, 0), colidx]
        bad = pos < 0
        if bad.any():
            out[bad] = nodes[ids[bad]].astype(np.float64) @ w
        return out

    qe_g = qe_dev[np.where(E_inv < ne_dev, E_inv, 0)]
    bad_e = E_inv >= ne_dev
    if bad_e.any():
        qe_g[bad_e] = edges[E[bad_e]].astype(np.float64) @ u1

    qg = globs.astype(np.float64) @ w1                    # [512]
    p_g = qg[U] + pa[:, 0] + cg
    p_n = _nd_dot(0, V, w3) + pa[:, 1] + cn
    p_e = (qe_g + _nd_dot(1, row[E], wr) + _nd_dot(2, col[E], wc)
           + pa[:, 2] + ce)

    actions_p = np.zeros(A_TOTAL, np.float64)
    actions_p[UA] = p_g
    actions_p[VA] = p_n
    actions_p[EA] = p_e

    # torch-style _norm: consecutive group ids starting at actions_batch[0]
    ab = actions_batch.astype(np.int64)
    changed = ab[1:] != ab[:-1]
    seg = int(ab[0]) + np.concatenate([[0], np.cumsum(changed)])
    if seg[0] >= 0 and seg[-1] < NUM_ACTIONS:
        agg = np.bincount(seg, weights=actions_p, minlength=NUM_ACTIONS)[:NUM_ACTIONS]
    else:  # jax segment_sum drops out-of-range ids
        agg = np.zeros(NUM_ACTIONS, np.float64)
        valid = (seg >= 0) & (seg < NUM_ACTIONS)
        np.add.at(agg, seg[valid], actions_p[valid])

    out = agg + float(pol_b.astype(np.float64)[0])
    return out.astype(np.float32)[:, None]


# revision 11
# speedup vs baseline: 3.1283x; 1.0049x over previous
"""Trainium2 Bass kernel for nn_Actions_block_14388140442036 (gnn_message_passing).

The reference network is entirely linear (no activations), so the output
    out = segment_sum(actions) @ pol_W + pol_b
collapses to per-effect scalars:
    p[j] = actions[j] @ pol_W  (a dot product against fused weight vectors)
followed by a scalar segment-sum.  Folding pol_W through each branch:

  glob branch:  p_g[i] = (globs @ w1)[U[i]]     + action_globs[i]. w2 + cg
  node branch:  p_n[i] = (nodes @ w3)[V[i]]     + action_nodes[i]. w4 + cn
  edge branch:  p_e[i] = (edges[E[i]] . u1) + (nodes @ wr)[row[E[i]]]
                        + (nodes @ wc)[col[E[i]]] + action_edges[i]. wv + ce

where  w1|w2 = glob_W @ pol_W,  w3|w4 = node_W @ pol_W,
       u1|u2 = e2_W @ pol_W,    wr|wv|wc = e1_W @ u2.

The device streams exactly the rows whose dots are needed, once each:
  * edges: only the UNIQUE rows referenced by E (~88.5k of 400k) are
    gathered on the host and streamed; duplicate effects share the dot.
  * nodes: only rows referenced by V | row[E] | col[E] (~95k of 100k)
    stream, each with three fused weight columns.
  * action features: all 100k effects, 48 features each.
Row capacities are padded to static shapes; in the (vanishingly unlikely)
event the live row count exceeds capacity, the overflow rows' dots are
computed on the host at full precision.

Device-side layout (per core, ~3.6MB): feature-major fp8 E3M4 streams
(nodes_fm [128, 11968], eg_fm [128, 11264], ap_fm [48, 12500]) with fp16
fused-weight vectors.  Feature-major means every 128-row group is directly
a valid matmul stationary operand ([K=feat, M=rows]); the PE computes all
seven dot columns (w3|wr|wc, u1, w2|w4|wv) with tiny moving operands and NO
transposes, no DVE work, no PSUM slab copies.  E3M4 (4 mantissa bits) keeps
every product exact against an fp16 weight with fp32 PSUM accumulation:
measured end-to-end rel err 1.02e-2 against the 2e-2 gate.  Dots accumulate
across the whole stream in three PSUM banks and drain once per stream via
an ACT downcast copy + one DMA, ordered so the stream whose transfer lands
last (eg) owns the smallest exposed drain chain.  The host does the tiny
fused-weight precompute, the scalar gathers and the segment sum.
"""

import numpy as np

import concourse.bacc as bacc
import concourse.mybir as mybir
import concourse.tile as tile
from concourse.bass_utils import run_bass_kernel_spmd

# ---- problem constants (hardcoded; kernel.py must be self-contained) ----
HID = 128
FEAT = 16
N_NODES = 100000
N_EDGES = 400000
N_PER = 100000
A_TOTAL = 300000
NUM_ACTIONS = 75000
N_CORES = 8

A_SH = N_PER // N_CORES      # 12500 action-effect rows per core
ND_SH = 11776                # compacted node rows per core (92*128)
EG_SH = 11136                # deduped edge rows per core (87*128)
ND_CAP = ND_SH * N_CORES     # 94208 >= |V u row[E] u col[E]| (93626 @ seed 0)
EG_CAP = EG_SH * N_CORES     # 89088 >= |unique(E)| (88489 @ seed 0)

# per-stream geometry: (rows, n_groups, tail_rows, [(chunk_cols, chunk_groups)])
AP_GEO = (A_SH, 98, 84, [(3200, 25), (3200, 25), (3200, 25), (2900, 23)])
ND_GEO = (ND_SH, 92, 128, [(3200, 25), (3200, 25), (3200, 25), (2176, 17)])
EG_GEO = (EG_SH, 87, 128, [(3200, 25), (3200, 25), (3200, 25), (1536, 12)])

F16 = mybir.dt.float16
F32 = mybir.dt.float32
F8 = mybir.dt.float8e3   # E3M4: 4 mantissa bits, range +/-15.5

_CACHE = {}


def _build_program(repeat=1):
    nc = bacc.Bacc("TRN2", target_bir_lowering=False, debug=False,
                   num_devices=N_CORES)

    nodes_in = nc.dram_tensor("nodes_fm", [HID, ND_SH], F8, kind="ExternalInput").ap()
    eg_in = nc.dram_tensor("eg_fm", [HID, EG_SH], F8, kind="ExternalInput").ap()
    ap_in = nc.dram_tensor("ap_fm", [3 * FEAT, A_SH], F8, kind="ExternalInput").ap()
    wts_in = nc.dram_tensor("wts_in", [128, 8], F16, kind="ExternalInput").ap()

    qn_out = nc.dram_tensor("qn_out", [128, 3 * ND_GEO[1]], F16, kind="ExternalOutput").ap()
    qg_out = nc.dram_tensor("qg_out", [128, EG_GEO[1]], F16, kind="ExternalOutput").ap()
    pa_out = nc.dram_tensor("pa_out", [128, 3 * AP_GEO[1]], F16, kind="ExternalOutput").ap()

    with tile.TileContext(nc) as tc:
        with (
            tc.tile_pool(name="wpool", bufs=1) as wpool,
            tc.tile_pool(name="spool", bufs=4) as spool,
            tc.tile_pool(name="opool", bufs=1) as opool,
            tc.tile_pool(name="pspool", bufs=1, space="PSUM") as pspool,
        ):
            # weights ride Pool-engine SWDGE: no slot on the shared HWDGE
            # generator, so the data streams start DMA-ing immediately
            wt = wpool.tile([128, 8], F16, tag="wt")
            nc.gpsimd.dma_start(wt[:], wts_in[:])

            qn_ps = pspool.tile([128, 3 * ND_GEO[1]], F32, tag="qn")
            qg_ps = pspool.tile([128, EG_GEO[1]], F32, tag="qg")
            pa_ps = pspool.tile([128, 3 * AP_GEO[1]], F32, tag="pa")

            for _rep in range(repeat):
                # all stream DMAs issue up front, round-robin across the
                # three streams.  Order (ap, nd, eg) per round: the stream
                # whose final transfer lands last (eg) owns the exposed
                # drain chain, and its drain copy is the smallest.
                tiles = {"ap": [], "nd": [], "eg": []}
                offs = {"ap": 0, "nd": 0, "eg": 0}
                for k in range(4):
                    for key, parts, geo, src in (
                        ("ap", 3 * FEAT, AP_GEO, ap_in),
                        ("nd", 128, ND_GEO, nodes_in),
                        ("eg", 128, EG_GEO, eg_in),
                    ):
                        cols = geo[3][k][0]
                        t = spool.tile([parts, 3200], F8, tag=key)
                        c0 = offs[key]
                        nc.sync.dma_start(t[:, :cols], src[:, c0:c0 + cols])
                        tiles[key].append(t)
                        offs[key] += cols

                # one [K, 128] stationary + tiny moving matmul per group and
                # stream.  PE runs in program order, so within a chunk the
                # matmuls go stream-major in DMA arrival order (ap, nd, eg):
                # each stream's dots run as soon as its chunk lands, and the
                # pa/qn accumulators complete before the final eg transfer.
                g0s = {"ap": 0, "nd": 0, "eg": 0}
                for k in range(4):
                    for key, geo, ps, w_lo, w_hi, wk, wd in (
                        ("ap", AP_GEO, pa_ps, 4, 7, 3 * FEAT, 3),
                        ("nd", ND_GEO, qn_ps, 0, 3, 128, 3),
                        ("eg", EG_GEO, qg_ps, 3, 4, 128, 1),
                    ):
                        ngroups, tail = geo[1], geo[2]
                        ng = geo[3][k][1]
                        g0 = g0s[key]
                        for j in range(ng):
                            g = g0 + j
                            m = tail if g == ngroups - 1 else 128
                            off = j * 128
                            nc.tensor.matmul(
                                ps[:m, wd * g:wd * g + wd],
                                tiles[key][k][:, off:off + m],
                                wt[:wk, w_lo:w_hi])
                        g0s[key] += ng

                # drains in stream-completion order; all copies on ACT
                # (idle), all out DMAs from SP (idle), so no drain blocks
                # another
                pa_sb = opool.tile([128, 3 * AP_GEO[1]], F16, tag="pasb")
                nc.scalar.copy(pa_sb[:], pa_ps[:])
                nc.sync.dma_start(pa_out[:], pa_sb[:])
                qn_sb = opool.tile([128, 3 * ND_GEO[1]], F16, tag="qnsb")
                nc.scalar.copy(qn_sb[:], qn_ps[:])
                nc.sync.dma_start(qn_out[:], qn_sb[:])
                qg_sb = opool.tile([128, EG_GEO[1]], F16, tag="qgsb")
                nc.scalar.copy(qg_sb[:], qg_ps[:])
                nc.sync.dma_start(qg_out[:], qg_sb[:])

    nc.compile()
    return nc


def _get_program():
    if "nc" not in _CACHE:
        _CACHE["nc"] = _build_program()
    return _CACHE["nc"]


def _unscr(a, ngroups, tail, w):
    """[128, ngroups*w] -> [(ngroups-1)*128 + tail, w]: group g spans cols
    w*g..w*g+w-1, row index within the stream is g*128 + partition."""
    a = a.astype(np.float64).reshape(128, ngroups, w)
    main = a[:, :ngroups - 1].transpose(1, 0, 2).reshape(-1, w)
    return np.concatenate([main, a[:tail, ngroups - 1]], axis=0)


def kernel(**inputs):
    inputs = {k: np.asarray(v) for k, v in inputs.items()}
    globs = inputs["globs"]
    nodes = inputs["nodes"]
    edges = inputs["edges"]
    action_globs = inputs["action_globs"]
    action_nodes = inputs["action_nodes"]
    action_edges = inputs["action_edges"]
    glob_W = inputs["glob_W"]; glob_b = inputs["glob_b"]
    node_W = inputs["node_W"]; node_b = inputs["node_b"]
    e1_W = inputs["e1_W"]; e1_b = inputs["e1_b"]
    e2_W = inputs["e2_W"]; e2_b = inputs["e2_b"]
    pol_W = inputs["pol_W"]; pol_b = inputs["pol_b"]
    row = inputs["row"]; col = inputs["col"]
    U = inputs["U"]; UA = inputs["UA"]; V = inputs["V"]; VA = inputs["VA"]
    E = inputs["E"]; EA = inputs["EA"]
    actions_batch = inputs["actions_batch"]

    # ---- fused weight vectors (float64 host math; fp16 on device) ----
    polW = pol_W.astype(np.float64)[:, 0]                 # [128]
    g_f = glob_W.astype(np.float64) @ polW                # [144]
    n_f = node_W.astype(np.float64) @ polW                # [144]
    e2_f = e2_W.astype(np.float64) @ polW                 # [256]
    u1, u2 = e2_f[:HID], e2_f[HID:]
    e1_f = e1_W.astype(np.float64) @ u2                   # [272]
    w1, w2 = g_f[:HID], g_f[HID:]
    w3, w4 = n_f[:HID], n_f[HID:]
    wr, wv, wc = e1_f[:HID], e1_f[HID:HID + FEAT], e1_f[HID + FEAT:]
    cg = float(glob_b.astype(np.float64) @ polW)
    cn = float(node_b.astype(np.float64) @ polW)
    ce = float(e2_b.astype(np.float64) @ polW + e1_b.astype(np.float64) @ u2)

    wts = np.zeros((128, 8), np.float16)
    wts[:, 0] = w3.astype(np.float16)
    wts[:, 1] = wr.astype(np.float16)
    wts[:, 2] = wc.astype(np.float16)
    wts[:, 3] = u1.astype(np.float16)
    wts[0:FEAT, 4] = w2.astype(np.float16)
    wts[FEAT:2 * FEAT, 5] = w4.astype(np.float16)
    wts[2 * FEAT:3 * FEAT, 6] = wv.astype(np.float16)

    # ---- host-side index compaction + fp8 E3M4 downcast + transpose to
    # feature-major (device groups become direct matmul stationaries) ----
    from ml_dtypes import float8_e3m4

    def _q8(x):
        return np.clip(x, -15.5, 15.5).astype(float8_e3m4)

    # unique referenced edge rows (dedup the E gather)
    Eu, E_inv = np.unique(E, return_inverse=True)         # Eu sorted
    ne_dev = min(len(Eu), EG_CAP)
    eg_dev = np.zeros((EG_CAP, HID), float8_e3m4)
    eg_dev[:ne_dev] = _q8(edges[Eu[:ne_dev]])

    # node rows actually referenced by any of the three gathers
    need = np.zeros(N_NODES, bool)
    need[V] = True
    need[row[E]] = True
    need[col[E]] = True
    nidx = np.flatnonzero(need)
    nn_dev = min(len(nidx), ND_CAP)
    nodes_dev = np.zeros((ND_CAP, HID), float8_e3m4)
    nodes_dev[:nn_dev] = _q8(nodes[nidx[:nn_dev]])
    newpos = np.full(N_NODES, -1, np.int64)
    newpos[nidx[:nn_dev]] = np.arange(nn_dev)

    ap8 = _q8(np.concatenate(
        [action_globs, action_nodes, action_edges], axis=1))

    nc = _get_program()
    in_maps = []
    for c in range(N_CORES):
        in_maps.append({
            "nodes_fm": np.ascontiguousarray(nodes_dev[c * ND_SH:(c + 1) * ND_SH].T),
            "eg_fm": np.ascontiguousarray(eg_dev[c * EG_SH:(c + 1) * EG_SH].T),
            "ap_fm": np.ascontiguousarray(ap8[c * A_SH:(c + 1) * A_SH].T),
            "wts_in": wts,
        })
    res = run_bass_kernel_spmd(nc, in_maps, core_ids=list(range(N_CORES)))

    qe_dev = np.empty(EG_CAP, np.float64)                 # unique-edge . u1
    qn3 = np.empty((ND_CAP, 3), np.float64)               # compacted node dots
    pa = np.empty((N_PER, 3), np.float64)
    for c in range(N_CORES):
        r = res.results[c]
        qn3[c * ND_SH:(c + 1) * ND_SH] = _unscr(r["qn_out"], ND_GEO[1], ND_GEO[2], 3)
        qe_dev[c * EG_SH:(c + 1) * EG_SH] = _unscr(r["qg_out"], EG_GEO[1], EG_GEO[2], 1)[:, 0]
        pa[c * A_SH:(c + 1) * A_SH] = _unscr(r["pa_out"], AP_GEO[1], AP_GEO[2], 3)

    # ---- host: gathers (with full-precision fallback for any rows beyond
    # the padded device capacity), scatter into action slots, segment sum ----
    def _nd_dot(colidx, ids, w):
        pos = newpos[ids]
        out = qn3[np.where(pos >= 0, pos, 0), colidx]
        bad = pos < 0
        if bad.any():
            out[bad] = nodes[ids[bad]].astype(np.float64) @ w
        return out

    qe_g = qe_dev[np.where(E_inv < ne_dev, E_inv, 0)]
    bad_e = E_inv >= ne_dev
    if bad_e.any():
        qe_g[bad_e] = edges[E[bad_e]].astype(np.float64) @ u1

    qg = globs.astype(np.float64) @ w1                    # [512]
    p_g = qg[U] + pa[:, 0] + cg
    p_n = _nd_dot(0, V, w3) + pa[:, 1] + cn
    p_e = (qe_g + _nd_dot(1, row[E], wr) + _nd_dot(2, col[E], wc)
           + pa[:, 2] + ce)

    actions_p = np.zeros(A_TOTAL, np.float64)
    actions_p[UA] = p_g
    actions_p[VA] = p_n
    actions_p[EA] = p_e

    # torch-style _norm: consecutive group ids starting at actions_batch[0]
    ab = actions_batch.astype(np.int64)
    changed = ab[1:] != ab[:-1]
    seg = int(ab[0]) + np.concatenate([[0], np.cumsum(changed)])
    if seg[0] >= 0 and seg[-1] < NUM_ACTIONS:
        agg = np.bincount(seg, weights=actions_p, minlength=NUM_ACTIONS)[:NUM_ACTIONS]
    else:  # jax segment_sum drops out-of-range ids
        agg = np.zeros(NUM_ACTIONS, np.float64)
        valid = (seg >= 0) & (seg < NUM_ACTIONS)
        np.add.at(agg, seg[valid], actions_p[valid])

    out = agg + float(pol_b.astype(np.float64)[0])
    return out.astype(np.float32)[:, None]
